# revision 27
# baseline (speedup 1.0000x reference)
"""Two-layer GAT (single-head, PyG-style) + link predictor on 8 TRN2 NeuronCores.

v2 strategy (memory-regime). Key observation: the second GAT layer's output h2
is consumed ONLY through four linear functionals per node — es2 = h2p@a_s2,
ed2 = h2p@a_d2, z0 = h2@wl0, z1 = h2@wl1 (wl0/wl1 = halves of the link weight
column). By linearity of the edge-softmax aggregation, layer-2 aggregation
therefore reduces to aggregating per-edge SCALARS:
    z0[dst] = (sum_e alpha_e * z0p[src_e]) / denom + b2.wl0
so launch 4 needs NO feature gather at all: the per-edge scalar operands
(z0p[src], z1p[src], 1) are host-expanded index-space arrays (same class as
the esx/edx expansions the v1 kernel already used), and the one-hot matmul
scatters them per destination window with a 4-wide rhs.

Launches:
  L1 p1 : h1p = X@W1 (+ folded es1/ed1 cols)        [unchanged from v1]
  L2 a1 : edge-softmax aggregate of h1p rows via per-tile indirect row
          gathers + one-hot PE matmuls; h1r = relu(agg+b1)   [unchanged]
  L3 p2 : h2p-projection folds extended to SIX columns:
          [W2 | W2@a_s2 | W2@a_d2 | W2@wl0 | W2@wl1]; outputs only the four
          per-node scalars (h2p itself never materializes in DRAM).
  L4 az : scalar aggregation (rhs = [z0x, z1x, 1, 0] per edge slot) ->
          z0/z1 per node. No gathers, PE cost (128+4) cols per tile.
  L5 lk : z = sigmoid(z0[m0] + z1[m1] + bl) on host-arranged index pairs.
"""
import time

import numpy as np

import concourse.bass as bass
import concourse.mybir as mybir
import concourse.tile as tile
from concourse import bacc
from concourse.bass_utils import run_bass_kernel_spmd

F32 = mybir.dt.float32
F16 = mybir.dt.float16
I32 = mybir.dt.int32

NCORES = 8
N, F_IN, H, C = 50000, 128, 256, 1
NS = N // NCORES            # 6250 nodes per shard
W = (NS + 127) // 128       # 49 windows per shard
NSP = W * 128               # 6272 padded slots
NEG = -1.0e30               # pad-edge sentinel (exp -> exactly 0)

LAST_EXEC_NS = {}           # launch name -> exec_time_ns (filled per kernel() call)
_PROG_CACHE = {}


# ----------------------------------------------------------------- host prep
NSPLIT = 32768          # int16 table split point for dma_gather
CH_TILES = 72           # max tiles per gather chunk (SBUF-bound)


def _prep_graph(edge_index):
    """Partition non-self edges by dst shard; within each dst window, edges
    are segregated by src half (A: src<32768, B: src>=32768) into separate
    128-slot tiles so a chunked int16 dma_gather can fetch whole tile groups.
    Common (max-over-cores) per-window tile counts wtA/wtB keep the program
    SPMD-shared. Edge slot s of a window region is (t, p) = (s//128, s%128)."""
    src = np.asarray(edge_index[0], np.int64)
    dst = np.asarray(edge_index[1], np.int64)

    core = dst // NS
    order = np.argsort(dst, kind="stable")
    src, dst, core = src[order], dst[order], core[order]

    e_src, e_dstloc = [], []
    for c in range(NCORES):
        m = core == c
        e_src.append(src[m])
        e_dstloc.append(dst[m] - c * NS)

    wtA = np.zeros(W, dtype=np.int64)
    wtB = np.zeros(W, dtype=np.int64)
    for c in range(NCORES):
        win = e_dstloc[c] // 128
        isA = e_src[c] < NSPLIT
        cntA = np.bincount(win[isA], minlength=W)
        cntB = np.bincount(win[~isA], minlength=W)
        wtA = np.maximum(wtA, (cntA + 127) // 128)
        wtB = np.maximum(wtB, (cntB + 127) // 128)
    wt = wtA + wtB
    T = int(wt.sum())
    wstart = np.concatenate([[0], np.cumsum(wt)]).astype(np.int64)

    srcs = np.zeros((NCORES, 128, T), dtype=np.int32)
    dstg = np.zeros((NCORES, 128, T), dtype=np.int32)
    dstf = np.full((NCORES, 128, T), -1.0, dtype=np.float32)
    kind = np.ones((NCORES, 128, T), dtype=np.int8)      # 0 real 1 pad
    # default pad sources: A tiles -> row 0, B tiles -> row NSPLIT
    for w in range(W):
        t0 = int(wstart[w])
        srcs[:, :, t0 + int(wtA[w]):t0 + int(wt[w])] = NSPLIT

    for c in range(NCORES):
        win = e_dstloc[c] // 128
        isA = e_src[c] < NSPLIT
        for w in range(W):
            t0 = int(wstart[w])
            for half, toff, flag in ((0, t0, True), (1, t0 + int(wtA[w]), False)):
                m = (win == w) & (isA == flag)
                s = e_src[c][m]
                dl = e_dstloc[c][m]
                n_e = len(s)
                sl = np.arange(n_e)
                tt, pp = toff + sl // 128, sl % 128
                srcs[c, pp, tt] = s
                dstg[c, pp, tt] = (dl + c * NS).astype(np.int32)
                dstf[c, pp, tt] = (dl - 128 * w).astype(np.float32)
                kind[c, pp, tt] = 0

    # chunk consecutive windows for the gather calls
    chunks = []
    cur = []
    cur_tiles = 0
    for w in range(W):
        tw = int(wt[w])
        if cur and cur_tiles + tw > CH_TILES:
            chunks.append(cur)
            cur, cur_tiles = [], 0
        cur.append(w)
        cur_tiles += tw
    if cur:
        chunks.append(cur)

    # per-chunk tile lists (global tile ids) for A and B calls + per-tile blk
    chmeta = []          # (wins, tilesA, tilesB) ; blk of tile = position
    blk_of = np.zeros(T, dtype=np.int64)
    for wins in chunks:
        tA, tB = [], []
        for w in wins:
            t0 = int(wstart[w])
            tA.extend(range(t0, t0 + int(wtA[w])))
            tB.extend(range(t0 + int(wtA[w]), t0 + int(wt[w])))
        for i, t in enumerate(tA):
            blk_of[t] = i
        for i, t in enumerate(tB):
            blk_of[t] = len(tA) + i
        chmeta.append((wins, tA, tB))

    # host-built int16 gather index arrays (16-partition wrap, replicated x8)
    def wrap16(vals):
        nidx = len(vals)
        colsn = (nidx + 15) // 16
        arr = np.zeros((128, colsn), np.int16)
        s = np.arange(nidx)
        for grp in range(8):
            arr[grp * 16 + s % 16, s // 16] = vals
        return arr

    idxA, idxB = [], []
    for c in range(NCORES):
        pa, pb = [], []
        for wins, tA, tB in chmeta:
            if tA:
                pa.append(wrap16(srcs[c][:, tA].T.ravel()))
            if tB:
                pb.append(wrap16((srcs[c][:, tB].T.ravel() - NSPLIT)))
        idxA.append(np.concatenate(pa, axis=1) if pa else np.zeros((128, 1), np.int16))
        idxB.append(np.concatenate(pb, axis=1) if pb else np.zeros((128, 1), np.int16))

    return dict(srcs=srcs, dstg=dstg, dstf=dstf, kind=kind, wt=wt,
                wtA=wtA, wtB=wtB, T=T, wstart=wstart, chmeta=chmeta,
                blk_of=blk_of, idxA=idxA, idxB=idxB,
                scA=idxA[0].shape[1], scB=idxB[0].shape[1])


def _expand(es_full, ed_full, g, c):
    """Host halo expansion: per-edge es[src], ed[dst] (+sentinel for pads),
    and per-node self-loop es/ed in [128, W] layout."""
    esx = es_full[g["srcs"][c]].astype(np.float32)
    edx = ed_full[np.minimum(g["dstg"][c], N - 1)].astype(np.float32)
    pad = g["kind"][c] == 1
    esx[pad] = NEG
    edx[pad] = 0.0
    nid = np.arange(NSP)
    nglob = np.minimum(c * NS + nid, N - 1)
    ess = np.where(nid < NS, es_full[nglob], 0.0).astype(np.float32)
    eds = np.where(nid < NS, ed_full[nglob], 0.0).astype(np.float32)
    return esx, edx, ess.reshape(W, 128).T.copy(), eds.reshape(W, 128).T.copy()


# ------------------------------------------------------------- bass programs
def _build_proj(kc, d_out, link_cols):
    """Projection per 128-node window: psum = x @ [W | W@a_s | W@a_d (|W@wl0
    |W@wl1)].  Inputs: xT fp16 [kc, W, 128, 128], Wm fp16 [kc*128, d_out],
    asr/adr (w0r/w1r) fp32 [128, d_out].
    Outputs: h16 [NSP, d_out+1] fp16 (feat + 1.0 col; omitted if link_cols),
    es/ed (z0/z1) [128, W] f32."""
    nf = 4 if link_cols else 2
    nc = bacc.Bacc(num_devices=NCORES)
    xT = nc.dram_tensor("xT", [kc, W, 128, 128], F16, kind="ExternalInput").ap()
    Wm = nc.dram_tensor("Wm", [kc * 128, d_out], F16, kind="ExternalInput").ap()
    folds = []
    for j, nm in enumerate(["asr", "adr", "w0r", "w1r"][:nf]):
        folds.append(nc.dram_tensor(nm, [128, d_out], F32, kind="ExternalInput").ap())
    if not link_cols:
        h16 = nc.dram_tensor("h16", [NSP, d_out + 1], F16, kind="ExternalOutput").ap()
    cols = nc.dram_tensor("cols", [128, nf * W], F32, kind="ExternalOutput").ap()

    with tile.TileContext(nc) as tc:
        with (
            tc.tile_pool(name="const", bufs=1) as cpool,
            tc.tile_pool(name="x", bufs=6) as xpool,
            tc.tile_pool(name="o", bufs=4) as opool,
            tc.tile_pool(name="ps", bufs=4, space="PSUM") as pspool,
            tc.tile_pool(name="sc", bufs=4) as scpool,
        ):
            fsb = []
            for j in range(nf):
                fb = cpool.tile([128, d_out], F32, tag=f"f{j}")
                nc.sync.dma_start(out=fb[:], in_=folds[j])
                fsb.append(fb)
            obuf = cpool.tile([128, nf * W], F32, name="obuf")

            wsb = []
            for k in range(kc):
                wk = cpool.tile([128, d_out + nf], F16, tag=f"w{k}")
                nc.sync.dma_start(
                    out=wk[:, 0:d_out], in_=Wm[128 * k:128 * (k + 1), :]
                )
                for j in range(nf):
                    scr = scpool.tile([128, d_out], F32, tag=f"wf{j}")
                    nc.vector.tensor_tensor(
                        out=scr[:], in0=wk[:, 0:d_out], in1=fsb[j][:],
                        op=mybir.AluOpType.mult,
                    )
                    wcol = scpool.tile([128, 1], F32, tag=f"wc{j}")
                    nc.vector.reduce_sum(
                        out=wcol[:], in_=scr[:], axis=mybir.AxisListType.X
                    )
                    nc.vector.tensor_copy(
                        out=wk[:, d_out + j:d_out + j + 1], in_=wcol[:]
                    )
                wsb.append(wk)

            for w in range(W):
                ps = pspool.tile([128, d_out + nf], F32, space="PSUM")
                for k in range(kc):
                    xt = xpool.tile([128, 128], F16)
                    nc.sync.dma_start(out=xt[:], in_=xT[k, w])
                    nc.tensor.matmul(
                        out=ps[:], lhsT=xt[:], rhs=wsb[k][:],
                        start=(k == 0), stop=(k == kc - 1),
                    )
                if not link_cols:
                    ht = opool.tile([128, d_out + 1], F16)
                    nc.scalar.copy(out=ht[:, 0:d_out], in_=ps[:, 0:d_out])
                    nc.vector.memset(ht[:, d_out:d_out + 1], 1.0)
                    nc.sync.dma_start(out=h16[128 * w:128 * (w + 1), :], in_=ht[:])
                nc.vector.tensor_copy(
                    out=obuf[:, nf * w:nf * (w + 1)],
                    in_=ps[:, d_out:d_out + nf],
                )
            nc.sync.dma_start(out=cols[:], in_=obuf[:])
    nc.compile()
    return nc


def _softmax_weights(nc, cpool, es_t, ed_t, cols, tagp):
    lg = cpool.tile([128, cols], F32, tag=f"lg{tagp}", name=f"lg{tagp}")
    nc.vector.tensor_tensor(
        out=lg[:], in0=es_t[:], in1=ed_t[:], op=mybir.AluOpType.add
    )
    lg2 = cpool.tile([128, cols], F32, tag=f"lg2{tagp}", name=f"lg2{tagp}")
    nc.vector.tensor_scalar_mul(out=lg2[:], in0=lg[:], scalar1=0.2)
    nc.vector.tensor_tensor(
        out=lg[:], in0=lg[:], in1=lg2[:], op=mybir.AluOpType.max
    )
    p = cpool.tile([128, cols], F32, tag=f"p{tagp}", name=f"p{tagp}")
    nc.scalar.activation(
        out=p[:], in_=lg[:], func=mybir.ActivationFunctionType.Exp
    )
    return p


def _build_agg(d, g, relu):
    """Layer-1 aggregation. Source rows are fetched with chunked int16
    dma_gather calls (table split at NSPLIT; one call per chunk per half,
    ~7.4ns/row Q7 emission vs ~11ns/row for per-tile INDIRECT1D), then
    scattered per dst window with one-hot PE matmuls.
    Output ho: [NSP, d] fp16 (normalized aggregate + bias (+relu))."""
    wt, wtA, wstart = g["wt"], g["wtA"], g["wstart"]
    chmeta, blk_of = g["chmeta"], g["blk_of"]
    T = int(sum(wt))
    ELEM = 384
    nc = bacc.Bacc(num_devices=NCORES)
    tabA = nc.dram_tensor("tabA", [NSPLIT, ELEM], F16, kind="ExternalInput").ap()
    tabB = nc.dram_tensor("tabB", [N - NSPLIT, ELEM], F16, kind="ExternalInput").ap()
    idxA = nc.dram_tensor("idxA", [128, g["scA"]], mybir.dt.int16,
                          kind="ExternalInput").ap()
    idxB = nc.dram_tensor("idxB", [128, g["scB"]], mybir.dt.int16,
                          kind="ExternalInput").ap()
    selftab = nc.dram_tensor("selftab", [NSP, d + 1], F16, kind="ExternalInput").ap()
    vrep = nc.dram_tensor("vrep", [128, 4 * d], F16, kind="ExternalInput").ap()
    dstf = nc.dram_tensor("dstf", [128, T], F32, kind="ExternalInput").ap()
    esx = nc.dram_tensor("esx", [128, T], F32, kind="ExternalInput").ap()
    edx = nc.dram_tensor("edx", [128, T], F32, kind="ExternalInput").ap()
    esself = nc.dram_tensor("esself", [128, W], F32, kind="ExternalInput").ap()
    edself = nc.dram_tensor("edself", [128, W], F32, kind="ExternalInput").ap()
    iota = nc.dram_tensor("iota", [128, 128], F32, kind="ExternalInput").ap()
    iotac = nc.dram_tensor("iotac", [128, 1], F32, kind="ExternalInput").ap()
    br = nc.dram_tensor("br", [128, d], F32, kind="ExternalInput").ap()
    cols = nc.dram_tensor("cols", [128, 4 * W], F32, kind="ExternalOutput").ap()

    with tile.TileContext(nc) as tc:
        with (
            tc.tile_pool(name="const", bufs=1) as cpool,
            tc.tile_pool(name="g", bufs=2) as gpool,
            tc.tile_pool(name="sf", bufs=4) as sfpool,
            tc.tile_pool(name="s", bufs=8) as spool,
            tc.tile_pool(name="o", bufs=3) as opool,
            tc.tile_pool(name="cl", bufs=6) as clpool,
            tc.tile_pool(name="ps", bufs=4, space="PSUM") as pspool,
        ):
            idxAs = cpool.tile([128, g["scA"]], mybir.dt.int16)
            nc.sync.dma_start(out=idxAs[:], in_=idxA[:])
            idxBs = cpool.tile([128, g["scB"]], mybir.dt.int16)
            nc.sync.dma_start(out=idxBs[:], in_=idxB[:])
            dsts = cpool.tile([128, T], F32)
            nc.sync.dma_start(out=dsts[:], in_=dstf[:])
            esxs = cpool.tile([128, T], F32)
            nc.sync.dma_start(out=esxs[:], in_=esx[:])
            edxs = cpool.tile([128, T], F32)
            nc.sync.dma_start(out=edxs[:], in_=edx[:])
            esss = cpool.tile([128, W], F32)
            nc.sync.dma_start(out=esss[:], in_=esself[:])
            edss = cpool.tile([128, W], F32)
            nc.sync.dma_start(out=edss[:], in_=edself[:])
            iosb = cpool.tile([128, 128], F32)
            nc.sync.dma_start(out=iosb[:], in_=iota[:])
            iocs = cpool.tile([128, 1], F32)
            nc.sync.dma_start(out=iocs[:], in_=iotac[:])
            brs = cpool.tile([128, d], F32)
            nc.sync.dma_start(out=brs[:], in_=br[:])
            vreps = cpool.tile([128, 4 * d], F16)
            nc.sync.dma_start(out=vreps[:], in_=vrep[:])
            colsb = cpool.tile([128, 4 * W], F32)

            p_all = _softmax_weights(nc, cpool, esxs, edxs, T, "e")
            p_self = _softmax_weights(nc, cpool, esss, edss, W, "s")

            caoff, cboff = 0, 0
            for wins, tA, tB in chmeta:
                nA, nB = len(tA), len(tB)
                gbuf = gpool.tile([128, nA + nB, ELEM], F16, tag="gb")
                if nA:
                    nc.gpsimd.dma_gather(
                        gbuf[:, 0:nA, :], tabA[:], idxAs[:, caoff:caoff + nA * 8],
                        nA * 128, nA * 128, ELEM, single_packet=False,
                    )
                    caoff += nA * 8
                if nB:
                    nc.gpsimd.dma_gather(
                        gbuf[:, nA:nA + nB, :], tabB[:],
                        idxBs[:, cboff:cboff + nB * 8],
                        nB * 128, nB * 128, ELEM, single_packet=False,
                    )
                    cboff += nB * 8
                for w in wins:
                    t = int(wstart[w])
                    ps = pspool.tile([128, d + 1], F32, space="PSUM")
                    st = sfpool.tile([128, d + 1], F16)
                    nc.sync.dma_start(
                        out=st[:], in_=selftab[128 * w:128 * (w + 1), :]
                    )
                    sd = spool.tile([128, 128], F16, tag="sdiag")
                    nc.vector.scalar_tensor_tensor(
                        out=sd[:], in0=iosb[:], scalar=iocs[:, :1],
                        in1=p_self[:, w:w + 1].to_broadcast([128, 128]),
                        op0=mybir.AluOpType.is_equal, op1=mybir.AluOpType.mult,
                    )
                    nc.tensor.matmul(
                        out=ps[:], lhsT=sd[:], rhs=st[:],
                        start=True, stop=(int(wt[w]) == 0),
                    )
                    for i in range(int(wt[w])):
                        blk = int(blk_of[t])
                        sp = spool.tile([128, 128], F16, tag="sedge")
                        nc.vector.scalar_tensor_tensor(
                            out=sp[:], in0=iosb[:], scalar=dsts[:, t:t + 1],
                            in1=p_all[:, t:t + 1].to_broadcast([128, 128]),
                            op0=mybir.AluOpType.is_equal, op1=mybir.AluOpType.mult,
                        )
                        nc.tensor.matmul(
                            out=ps[:], lhsT=sp[:], rhs=gbuf[:, blk, 0:d + 1],
                            start=False, stop=(i == int(wt[w]) - 1),
                        )
                        t += 1
                    _agg_epilogue(nc, clpool, opool, ps, brs, vreps, colsb, d, w)
            nc.sync.dma_start(out=cols[:], in_=colsb[:])
    nc.compile()
    return nc


def _agg_epilogue(nc, clpool, opool, ps, brs, vreps, colsb, d, w):
    """h1r = relu(agg/denom + b1); then the fused layer-2 projection columns
    es2/ed2/z0p/z1p = h1r . (W2@a_s2 | W2@a_d2 | W2@wl0 | W2@wl1)."""
    rec = clpool.tile([128, 1], F32)
    nc.vector.reciprocal(rec[:], ps[:, d:d + 1])
    ot = opool.tile([128, d], F32)
    nc.vector.scalar_tensor_tensor(
        out=ot[:], in0=ps[:, 0:d], scalar=rec[:, :1], in1=brs[:],
        op0=mybir.AluOpType.mult, op1=mybir.AluOpType.add,
    )
    ot16 = opool.tile([128, d], F16, tag="o16")
    nc.vector.tensor_scalar_max(out=ot16[:], in0=ot[:], scalar1=0.0)
    for j in range(4):
        scr = opool.tile([128, d], F32, tag=f"scr{j}")
        nc.vector.tensor_tensor(
            out=scr[:], in0=ot16[:], in1=vreps[:, d * j:d * (j + 1)],
            op=mybir.AluOpType.mult,
        )
        nc.vector.reduce_sum(
            out=colsb[:, 4 * w + j:4 * w + j + 1], in_=scr[:],
            axis=mybir.AxisListType.X,
        )


def _build_agg_z(wt):
    """Layer-2 scalar aggregation: gather-free. Per edge slot the rhs operands
    are host-expanded scalars [z0p[src], z1p[src], 1, 0]; the one-hot matmul
    scatters them by dst; epilogue: z_j[dst] = ps[:, j]/ps[:, 2] + c_j.
    One-hot lhsT tiles alternate between DVE is_eq builds and ACT-scaled
    static 0/1 tiles streamed from DRAM, halving the per-tile serial cost."""
    T = int(sum(wt))
    nc = bacc.Bacc(num_devices=NCORES)
    zin = nc.dram_tensor("zin", [128, T, 4], F16, kind="ExternalInput").ap()
    selfz = nc.dram_tensor("selfz", [128, W, 4], F16, kind="ExternalInput").ap()
    oh = nc.dram_tensor("oh", [128, T, 128], F16, kind="ExternalInput").ap()
    dstf = nc.dram_tensor("dstf", [128, T], F32, kind="ExternalInput").ap()
    esx = nc.dram_tensor("esx", [128, T], F32, kind="ExternalInput").ap()
    edx = nc.dram_tensor("edx", [128, T], F32, kind="ExternalInput").ap()
    esself = nc.dram_tensor("esself", [128, W], F32, kind="ExternalInput").ap()
    edself = nc.dram_tensor("edself", [128, W], F32, kind="ExternalInput").ap()
    iota = nc.dram_tensor("iota", [128, 128], F32, kind="ExternalInput").ap()
    iotac = nc.dram_tensor("iotac", [128, 1], F32, kind="ExternalInput").ap()
    c01 = nc.dram_tensor("c01", [128, 2], F32, kind="ExternalInput").ap()
    zo = nc.dram_tensor("zo", [128, 2 * W], F32, kind="ExternalOutput").ap()

    with tile.TileContext(nc) as tc:
        with (
            tc.tile_pool(name="const", bufs=1) as cpool,
            tc.tile_pool(name="s", bufs=8) as spool,
            tc.tile_pool(name="obh", bufs=3) as ohpool,
            tc.tile_pool(name="cl", bufs=6) as clpool,
            tc.tile_pool(name="ps", bufs=4, space="PSUM") as pspool,
        ):
            zins = cpool.tile([128, T, 4], F16)
            nc.sync.dma_start(out=zins[:], in_=zin[:])
            selfzs = cpool.tile([128, W, 4], F16)
            nc.sync.dma_start(out=selfzs[:], in_=selfz[:])
            dsts = cpool.tile([128, T], F32)
            nc.sync.dma_start(out=dsts[:], in_=dstf[:])
            esxs = cpool.tile([128, T], F32)
            nc.sync.dma_start(out=esxs[:], in_=esx[:])
            edxs = cpool.tile([128, T], F32)
            nc.sync.dma_start(out=edxs[:], in_=edx[:])
            esss = cpool.tile([128, W], F32)
            nc.sync.dma_start(out=esss[:], in_=esself[:])
            edss = cpool.tile([128, W], F32)
            nc.sync.dma_start(out=edss[:], in_=edself[:])
            iosb = cpool.tile([128, 128], F32)
            nc.sync.dma_start(out=iosb[:], in_=iota[:])
            iocs = cpool.tile([128, 1], F32)
            nc.sync.dma_start(out=iocs[:], in_=iotac[:])
            c01s = cpool.tile([128, 2], F32)
            nc.sync.dma_start(out=c01s[:], in_=c01[:])
            zob = cpool.tile([128, 2 * W], F32)

            p_all = _softmax_weights(nc, cpool, esxs, edxs, T, "e")
            p_self = _softmax_weights(nc, cpool, esss, edss, W, "s")

            t = 0
            ohb, ohb_base = None, -1
            for w in range(W):
                ps = pspool.tile([128, 4], F32, space="PSUM")
                sd = spool.tile([128, 128], F16, tag="sdiag")
                nc.vector.scalar_tensor_tensor(
                    out=sd[:], in0=iosb[:], scalar=iocs[:, :1],
                    in1=p_self[:, w:w + 1].to_broadcast([128, 128]),
                    op0=mybir.AluOpType.is_equal, op1=mybir.AluOpType.mult,
                )
                nc.tensor.matmul(
                    out=ps[:], lhsT=sd[:], rhs=selfzs[:, w, 0:4],
                    start=True, stop=(int(wt[w]) == 0),
                )
                for i in range(int(wt[w])):
                    if t % 3 == 0:
                        # ACT path: static 0/1 tile (batch-loaded) scaled by p
                        if ohb is None or t >= ohb_base + 8:
                            ohb = ohpool.tile([128, 8, 128], F16, tag="ohb")
                            nb = min(8, T - t)
                            nc.sync.dma_start(
                                out=ohb[:, 0:nb, :], in_=oh[:, t:t + nb, :]
                            )
                            ohb_base = t
                        sp = spool.tile([128, 128], F16, tag="sedge")
                        nc.scalar.mul(
                            out=sp[:], in_=ohb[:, t - ohb_base, :],
                            mul=p_all[:, t:t + 1],
                        )
                    else:
                        sp = spool.tile([128, 128], F16, tag="sedge")
                        nc.vector.scalar_tensor_tensor(
                            out=sp[:], in0=iosb[:], scalar=dsts[:, t:t + 1],
                            in1=p_all[:, t:t + 1].to_broadcast([128, 128]),
                            op0=mybir.AluOpType.is_equal, op1=mybir.AluOpType.mult,
                        )
                    nc.tensor.matmul(
                        out=ps[:], lhsT=sp[:], rhs=zins[:, t, 0:4],
                        start=False, stop=(i == int(wt[w]) - 1),
                    )
                    t += 1
                rec = clpool.tile([128, 1], F32)
                nc.vector.reciprocal(rec[:], ps[:, 2:3])
                nc.vector.scalar_tensor_tensor(
                    out=zob[:, 2 * w:2 * w + 2], in0=ps[:, 0:2],
                    scalar=rec[:, :1], in1=c01s[:, 0:2],
                    op0=mybir.AluOpType.mult, op1=mybir.AluOpType.add,
                )
            nc.sync.dma_start(out=zo[:], in_=zob[:])
    nc.compile()
    return nc


def _build_link2(pt):
    """z = sigmoid(z0x + z1x + bl) for pt*128 host-arranged pairs."""
    nc = bacc.Bacc(num_devices=NCORES)
    z0x = nc.dram_tensor("z0x", [128, pt], F32, kind="ExternalInput").ap()
    z1x = nc.dram_tensor("z1x", [128, pt], F32, kind="ExternalInput").ap()
    blr = nc.dram_tensor("blr", [128, 1], F32, kind="ExternalInput").ap()
    z = nc.dram_tensor("z", [128, pt], F32, kind="ExternalOutput").ap()

    with tile.TileContext(nc) as tc:
        with tc.tile_pool(name="c", bufs=1) as cpool:
            z0s = cpool.tile([128, pt], F32)
            nc.sync.dma_start(out=z0s[:], in_=z0x[:])
            z1s = cpool.tile([128, pt], F32)
            nc.sync.dma_start(out=z1s[:], in_=z1x[:])
            bls = cpool.tile([128, 1], F32)
            nc.sync.dma_start(out=bls[:], in_=blr[:])
            zs = cpool.tile([128, pt], F32, name="zs")
            nc.vector.tensor_tensor(
                out=zs[:], in0=z0s[:], in1=z1s[:], op=mybir.AluOpType.add
            )
            zsb = cpool.tile([128, pt], F32, name="zsb")
            nc.scalar.activation(
                out=zsb[:], in_=zs[:],
                func=mybir.ActivationFunctionType.Sigmoid, bias=bls[:, :1],
            )
            nc.sync.dma_start(out=z[:], in_=zsb[:])
    nc.compile()
    return nc


def _run(name, nc, in_maps, trace=True):
    last = None
    for attempt in range(3):
        try:
            res = run_bass_kernel_spmd(
                nc, in_maps, core_ids=list(range(NCORES)), trace=trace
            )
            LAST_EXEC_NS[name] = res.exec_time_ns
            return res.results
        except Exception as e:  # wedged-device retry (clears on re-attempt)
            last = e
            time.sleep(5)
    raise last


def _rep(v, n=128):
    return np.ascontiguousarray(np.broadcast_to(np.asarray(v, np.float32), (n, len(v))))


def _tile_xT(xfull_shards, kc, d_in):
    """list of [NSP, d_in] fp16 per core -> [NCORES, kc, W, 128, 128] fp16."""
    out = np.zeros((NCORES, kc, W, 128, 128), np.float16)
    for c in range(NCORES):
        xt = xfull_shards[c].T  # [d_in, NSP]
        for k in range(kc):
            blk = xt[128 * k:128 * (k + 1)].reshape(128, W, 128)
            out[c, k] = np.transpose(blk, (1, 0, 2))
    return out


# ------------------------------------------------------------------- kernel
def kernel(features, edge_index, mask, W1, a_src1, a_dst1, b1, W2, a_src2,
           a_dst2, b2, Wl, bl):
    features = np.asarray(features, np.float32)
    edge_index = np.asarray(edge_index, np.int32)
    mask = np.asarray(mask, np.int32)
    W1, W2, Wl = (np.asarray(a, np.float32) for a in (W1, W2, Wl))
    a_src1, a_dst1, b1 = (np.asarray(a, np.float32) for a in (a_src1, a_dst1, b1))
    a_src2, a_dst2, b2 = (np.asarray(a, np.float32) for a in (a_src2, a_dst2, b2))
    bl = np.asarray(bl, np.float32)
    wl0, wl1 = Wl[:F_IN, 0], Wl[F_IN:, 0]

    g = _prep_graph(edge_index)
    iota = np.ascontiguousarray(
        np.broadcast_to(np.arange(128, dtype=np.float32), (128, 128))
    )
    iotac = np.arange(128, dtype=np.float32).reshape(128, 1)

    key = (g["T"], tuple(int(x) for x in g["wt"]))
    if key not in _PROG_CACHE:
        _PROG_CACHE[key] = dict(
            p1=_build_proj(1, H, link_cols=False),
            a1=_build_agg(H, g, relu=True),
            az=_build_agg_z(g["wt"]),
            lk=_build_link2((10000 // NCORES + 127) // 128),
        )
    progs = _PROG_CACHE[key]

    # ---- L1: H1 = X @ W1 (sharded), es1/ed1
    xsh = []
    for c in range(NCORES):
        xs = np.zeros((NSP, F_IN), np.float16)
        xs[:NS] = features[c * NS:(c + 1) * NS]
        xsh.append(xs)
    xT1 = _tile_xT(xsh, 1, F_IN)
    W1h = W1.astype(np.float16)
    r1 = _run("p1", progs["p1"], [
        dict(xT=xT1[c], Wm=W1h, asr=_rep(a_src1), adr=_rep(a_dst1))
        for c in range(NCORES)
    ])
    H1e = np.concatenate([r1[c]["h16"][:NS] for c in range(NCORES)])   # [N, H+1] f16
    es1 = np.concatenate([r1[c]["cols"][:, 0::2].T.ravel()[:NS] for c in range(NCORES)])
    ed1 = np.concatenate([r1[c]["cols"][:, 1::2].T.ravel()[:NS] for c in range(NCORES)])

    # ---- L2: aggregate layer 1, then fused in-epilogue layer-2 projection:
    # cols = [es2 | ed2 | z0p | z1p] per local node (h1r never leaves device)
    b1r = _rep(b1)
    T1 = np.zeros((N, 384), np.float16)
    T1[:, 0:H + 1] = H1e
    vfold = np.stack([W2 @ a_src2, W2 @ a_dst2, W2 @ wl0, W2 @ wl1])   # [4, 256]
    vrep = np.ascontiguousarray(np.broadcast_to(
        vfold.reshape(1, 4 * H), (128, 4 * H))).astype(np.float16)
    ins2 = []
    for c in range(NCORES):
        esx, edx, ess, eds = _expand(es1, ed1, g, c)
        st = np.zeros((NSP, H + 1), np.float16)
        st[:NS] = H1e[c * NS:(c + 1) * NS]
        ins2.append(dict(tabA=T1[:NSPLIT], tabB=T1[NSPLIT:],
                         idxA=g["idxA"][c], idxB=g["idxB"][c],
                         selftab=st, vrep=vrep, dstf=g["dstf"][c],
                         esx=esx, edx=edx, esself=ess, edself=eds,
                         iota=iota, iotac=iotac, br=b1r))
    r2 = _run("a1", progs["a1"], ins2)
    es2 = np.concatenate([r2[c]["cols"][:, 0::4].T.ravel()[:NS] for c in range(NCORES)])
    ed2 = np.concatenate([r2[c]["cols"][:, 1::4].T.ravel()[:NS] for c in range(NCORES)])
    z0p = np.concatenate([r2[c]["cols"][:, 2::4].T.ravel()[:NS] for c in range(NCORES)])
    z1p = np.concatenate([r2[c]["cols"][:, 3::4].T.ravel()[:NS] for c in range(NCORES)])

    # ---- L4: scalar aggregation -> z0/z1 per node
    c0 = float(b2 @ wl0)
    c1 = float(b2 @ wl1)
    c01 = np.ascontiguousarray(
        np.broadcast_to(np.array([c0, c1], np.float32), (128, 2))
    )
    ins4 = []
    for c in range(NCORES):
        esx, edx, ess, eds = _expand(es2, ed2, g, c)
        real = (g["kind"][c] == 0)
        zin = np.zeros((128, g["T"], 4), np.float16)
        zin[:, :, 0] = np.where(real, z0p[g["srcs"][c]], 0.0)
        zin[:, :, 1] = np.where(real, z1p[g["srcs"][c]], 0.0)
        zin[:, :, 2] = real.astype(np.float16)
        nid = np.arange(NSP)
        nglob = np.minimum(c * NS + nid, N - 1)
        valid = (nid < NS)
        selfz = np.zeros((128, W, 4), np.float16)
        selfz[:, :, 0] = np.where(valid, z0p[nglob], 0.0).reshape(W, 128).T
        selfz[:, :, 1] = np.where(valid, z1p[nglob], 0.0).reshape(W, 128).T
        selfz[:, :, 2] = valid.astype(np.float16).reshape(W, 128).T
        ohc = np.zeros((128, g["T"], 128), np.float16)
        pp, tt = np.nonzero(real)
        ohc[pp, tt, g["dstf"][c][pp, tt].astype(np.int64)] = 1.0
        ins4.append(dict(zin=zin, selfz=selfz, oh=ohc, dstf=g["dstf"][c],
                         esx=esx, edx=edx, esself=ess, edself=eds,
                         iota=iota, iotac=iotac, c01=c01))
    r4 = _run("az", progs["az"], ins4)
    zoc = [r4[c]["zo"] for c in range(NCORES)]         # [128, 2W] f32 per core
    z0f = np.concatenate(
        [zoc[c][:, 0::2].T.ravel()[:NS] for c in range(NCORES)])
    z1f = np.concatenate(
        [zoc[c][:, 1::2].T.ravel()[:NS] for c in range(NCORES)])

    # ---- L5: z = sigmoid(z0[m0] + z1[m1] + bl)
    P = mask.shape[0]
    pc = P // NCORES
    pt = (pc + 127) // 128
    z0x = np.zeros((NCORES, 128, pt), np.float32)
    z1x = np.zeros((NCORES, 128, pt), np.float32)
    mT = mask.T
    for c in range(NCORES):
        s = np.arange(pc)
        z0x[c, s % 128, s // 128] = z0f[mT[0][c * pc:(c + 1) * pc]]
        z1x[c, s % 128, s // 128] = z1f[mT[1][c * pc:(c + 1) * pc]]
    blr = np.full((128, 1), float(bl[0]), np.float32)
    r5 = _run("lk", progs["lk"], [
        dict(z0x=z0x[c], z1x=z1x[c], blr=blr)
        for c in range(NCORES)
    ])
    out = np.zeros((P, 1), np.float32)
    for c in range(NCORES):
        s = np.arange(pc)
        out[c * pc:(c + 1) * pc, 0] = r5[c]["z"][s % 128, s // 128]

    tot = sum(v for v in LAST_EXEC_NS.values() if v)
    print(f"kernel launches ns: {LAST_EXEC_NS} total {tot}")
    return out


# revision 34
# speedup vs baseline: 1.0802x; 1.0802x over previous
"""Two-layer GAT (single-head, PyG-style) + link predictor on 8 TRN2 NeuronCores.

v2 strategy (memory-regime). Key observation: the second GAT layer's output h2
is consumed ONLY through four linear functionals per node — es2 = h2p@a_s2,
ed2 = h2p@a_d2, z0 = h2@wl0, z1 = h2@wl1 (wl0/wl1 = halves of the link weight
column). By linearity of the edge-softmax aggregation, layer-2 aggregation
therefore reduces to aggregating per-edge SCALARS:
    z0[dst] = (sum_e alpha_e * z0p[src_e]) / denom + b2.wl0
so launch 4 needs NO feature gather at all: the per-edge scalar operands
(z0p[src], z1p[src], 1) are host-expanded index-space arrays (same class as
the esx/edx expansions the v1 kernel already used), and the one-hot matmul
scatters them per destination window with a 4-wide rhs.

Launches:
  L1 p1 : h1p = X@W1 (+ folded es1/ed1 cols)        [unchanged from v1]
  L2 a1 : edge-softmax aggregate of h1p rows via per-tile indirect row
          gathers + one-hot PE matmuls; h1r = relu(agg+b1)   [unchanged]
  L3 p2 : h2p-projection folds extended to SIX columns:
          [W2 | W2@a_s2 | W2@a_d2 | W2@wl0 | W2@wl1]; outputs only the four
          per-node scalars (h2p itself never materializes in DRAM).
  L4 az : scalar aggregation (rhs = [z0x, z1x, 1, 0] per edge slot) ->
          z0/z1 per node. No gathers, PE cost (128+4) cols per tile.
  L5 lk : z = sigmoid(z0[m0] + z1[m1] + bl) on host-arranged index pairs.
"""
import time

import numpy as np

import concourse.bass as bass
import concourse.mybir as mybir
import concourse.tile as tile
from concourse import bacc
from concourse.bass_utils import run_bass_kernel_spmd

F32 = mybir.dt.float32
F16 = mybir.dt.float16
I32 = mybir.dt.int32

NCORES = 8
N, F_IN, H, C = 50000, 128, 256, 1
NS = N // NCORES            # 6250 nodes per shard
W = (NS + 127) // 128       # 49 windows per shard
NSP = W * 128               # 6272 padded slots
NEG = -1.0e30               # pad-edge sentinel (exp -> exactly 0)

LAST_EXEC_NS = {}           # launch name -> exec_time_ns (filled per kernel() call)
_PROG_CACHE = {}


# ----------------------------------------------------------------- host prep
NSPLIT = 32768          # int16 table split point for dma_gather
CH_TILES = 72           # max tiles per gather chunk (SBUF-bound)


def _prep_graph(edge_index):
    """Partition non-self edges by dst shard; within each dst window, edges
    are segregated by src half (A: src<32768, B: src>=32768) into separate
    128-slot tiles so a chunked int16 dma_gather can fetch whole tile groups.
    Common (max-over-cores) per-window tile counts wtA/wtB keep the program
    SPMD-shared. Edge slot s of a window region is (t, p) = (s//128, s%128)."""
    src = np.asarray(edge_index[0], np.int64)
    dst = np.asarray(edge_index[1], np.int64)

    core = dst // NS
    order = np.argsort(dst, kind="stable")
    src, dst, core = src[order], dst[order], core[order]

    e_src, e_dstloc = [], []
    for c in range(NCORES):
        m = core == c
        e_src.append(src[m])
        e_dstloc.append(dst[m] - c * NS)

    wtA = np.zeros(W, dtype=np.int64)
    wtB = np.zeros(W, dtype=np.int64)
    for c in range(NCORES):
        win = e_dstloc[c] // 128
        isA = e_src[c] < NSPLIT
        cntA = np.bincount(win[isA], minlength=W)
        cntB = np.bincount(win[~isA], minlength=W)
        wtA = np.maximum(wtA, (cntA + 127) // 128)
        wtB = np.maximum(wtB, (cntB + 127) // 128)
    wt = wtA + wtB
    T = int(wt.sum())
    wstart = np.concatenate([[0], np.cumsum(wt)]).astype(np.int64)

    srcs = np.zeros((NCORES, 128, T), dtype=np.int32)
    dstg = np.zeros((NCORES, 128, T), dtype=np.int32)
    dstf = np.full((NCORES, 128, T), -1.0, dtype=np.float32)
    kind = np.ones((NCORES, 128, T), dtype=np.int8)      # 0 real 1 pad
    # default pad sources: A tiles -> row 0, B tiles -> row NSPLIT
    for w in range(W):
        t0 = int(wstart[w])
        srcs[:, :, t0 + int(wtA[w]):t0 + int(wt[w])] = NSPLIT

    for c in range(NCORES):
        win = e_dstloc[c] // 128
        isA = e_src[c] < NSPLIT
        for w in range(W):
            t0 = int(wstart[w])
            for half, toff, flag in ((0, t0, True), (1, t0 + int(wtA[w]), False)):
                m = (win == w) & (isA == flag)
                s = e_src[c][m]
                dl = e_dstloc[c][m]
                n_e = len(s)
                sl = np.arange(n_e)
                tt, pp = toff + sl // 128, sl % 128
                srcs[c, pp, tt] = s
                dstg[c, pp, tt] = (dl + c * NS).astype(np.int32)
                dstf[c, pp, tt] = (dl - 128 * w).astype(np.float32)
                kind[c, pp, tt] = 0

    # chunk consecutive windows for the gather calls
    chunks = []
    cur = []
    cur_tiles = 0
    for w in range(W):
        tw = int(wt[w])
        if cur and cur_tiles + tw > CH_TILES:
            chunks.append(cur)
            cur, cur_tiles = [], 0
        cur.append(w)
        cur_tiles += tw
    if cur:
        chunks.append(cur)

    # per-chunk tile lists (global tile ids) for A and B calls + per-tile blk
    chmeta = []          # (wins, tilesA, tilesB) ; blk of tile = position
    blk_of = np.zeros(T, dtype=np.int64)
    for wins in chunks:
        tA, tB = [], []
        for w in wins:
            t0 = int(wstart[w])
            tA.extend(range(t0, t0 + int(wtA[w])))
            tB.extend(range(t0 + int(wtA[w]), t0 + int(wt[w])))
        for i, t in enumerate(tA):
            blk_of[t] = i
        for i, t in enumerate(tB):
            blk_of[t] = len(tA) + i
        chmeta.append((wins, tA, tB))

    # host-built int16 gather index arrays (16-partition wrap, replicated x8)
    def wrap16(vals):
        nidx = len(vals)
        colsn = (nidx + 15) // 16
        arr = np.zeros((128, colsn), np.int16)
        s = np.arange(nidx)
        for grp in range(8):
            arr[grp * 16 + s % 16, s // 16] = vals
        return arr

    idxA, idxB = [], []
    for c in range(NCORES):
        pa, pb = [], []
        for wins, tA, tB in chmeta:
            if tA:
                pa.append(wrap16(srcs[c][:, tA].T.ravel()))
            if tB:
                pb.append(wrap16((srcs[c][:, tB].T.ravel() - NSPLIT)))
        idxA.append(np.concatenate(pa, axis=1) if pa else np.zeros((128, 1), np.int16))
        idxB.append(np.concatenate(pb, axis=1) if pb else np.zeros((128, 1), np.int16))

    return dict(srcs=srcs, dstg=dstg, dstf=dstf, kind=kind, wt=wt,
                wtA=wtA, wtB=wtB, T=T, wstart=wstart, chmeta=chmeta,
                blk_of=blk_of, idxA=idxA, idxB=idxB,
                scA=idxA[0].shape[1], scB=idxB[0].shape[1])


def _expand(es_full, ed_full, g, c):
    """Host halo expansion: per-edge es[src], ed[dst] (+sentinel for pads),
    and per-node self-loop es/ed in [128, W] layout."""
    esx = es_full[g["srcs"][c]].astype(np.float32)
    edx = ed_full[np.minimum(g["dstg"][c], N - 1)].astype(np.float32)
    pad = g["kind"][c] == 1
    esx[pad] = NEG
    edx[pad] = 0.0
    nid = np.arange(NSP)
    nglob = np.minimum(c * NS + nid, N - 1)
    ess = np.where(nid < NS, es_full[nglob], 0.0).astype(np.float32)
    eds = np.where(nid < NS, ed_full[nglob], 0.0).astype(np.float32)
    return esx, edx, ess.reshape(W, 128).T.copy(), eds.reshape(W, 128).T.copy()


# ------------------------------------------------------------- bass programs
def _build_proj(kc, d_out, link_cols):
    """Projection per 128-node window: psum = x @ [W | W@a_s | W@a_d (|W@wl0
    |W@wl1)].  Inputs: xT fp16 [kc, W, 128, 128], Wm fp16 [kc*128, d_out],
    asr/adr (w0r/w1r) fp32 [128, d_out].
    Outputs: h16 [NSP, d_out+1] fp16 (feat + 1.0 col; omitted if link_cols),
    es/ed (z0/z1) [128, W] f32."""
    nf = 4 if link_cols else 2
    nc = bacc.Bacc(num_devices=NCORES)
    xT = nc.dram_tensor("xT", [kc, W, 128, 128], F16, kind="ExternalInput").ap()
    Wm = nc.dram_tensor("Wm", [kc * 128, d_out], F16, kind="ExternalInput").ap()
    folds = []
    for j, nm in enumerate(["asr", "adr", "w0r", "w1r"][:nf]):
        folds.append(nc.dram_tensor(nm, [128, d_out], F32, kind="ExternalInput").ap())
    if not link_cols:
        h16 = nc.dram_tensor("h16", [NSP, d_out + 1], F16, kind="ExternalOutput").ap()
    cols = nc.dram_tensor("cols", [128, nf * W], F32, kind="ExternalOutput").ap()

    with tile.TileContext(nc) as tc:
        with (
            tc.tile_pool(name="const", bufs=1) as cpool,
            tc.tile_pool(name="x", bufs=6) as xpool,
            tc.tile_pool(name="o", bufs=4) as opool,
            tc.tile_pool(name="ps", bufs=4, space="PSUM") as pspool,
            tc.tile_pool(name="sc", bufs=4) as scpool,
        ):
            fsb = []
            for j in range(nf):
                fb = cpool.tile([128, d_out], F32, tag=f"f{j}")
                nc.sync.dma_start(out=fb[:], in_=folds[j])
                fsb.append(fb)
            obuf = cpool.tile([128, nf * W], F32, name="obuf")

            wsb = []
            for k in range(kc):
                wk = cpool.tile([128, d_out + nf], F16, tag=f"w{k}")
                nc.sync.dma_start(
                    out=wk[:, 0:d_out], in_=Wm[128 * k:128 * (k + 1), :]
                )
                for j in range(nf):
                    scr = scpool.tile([128, d_out], F32, tag=f"wf{j}")
                    nc.vector.tensor_tensor(
                        out=scr[:], in0=wk[:, 0:d_out], in1=fsb[j][:],
                        op=mybir.AluOpType.mult,
                    )
                    wcol = scpool.tile([128, 1], F32, tag=f"wc{j}")
                    nc.vector.reduce_sum(
                        out=wcol[:], in_=scr[:], axis=mybir.AxisListType.X
                    )
                    nc.vector.tensor_copy(
                        out=wk[:, d_out + j:d_out + j + 1], in_=wcol[:]
                    )
                wsb.append(wk)

            for w in range(W):
                ps = pspool.tile([128, d_out + nf], F32, space="PSUM")
                for k in range(kc):
                    xt = xpool.tile([128, 128], F16)
                    nc.sync.dma_start(out=xt[:], in_=xT[k, w])
                    nc.tensor.matmul(
                        out=ps[:], lhsT=xt[:], rhs=wsb[k][:],
                        start=(k == 0), stop=(k == kc - 1),
                    )
                if not link_cols:
                    ht = opool.tile([128, d_out + 1], F16)
                    nc.scalar.copy(out=ht[:, 0:d_out], in_=ps[:, 0:d_out])
                    nc.vector.memset(ht[:, d_out:d_out + 1], 1.0)
                    nc.sync.dma_start(out=h16[128 * w:128 * (w + 1), :], in_=ht[:])
                nc.vector.tensor_copy(
                    out=obuf[:, nf * w:nf * (w + 1)],
                    in_=ps[:, d_out:d_out + nf],
                )
            nc.sync.dma_start(out=cols[:], in_=obuf[:])
    nc.compile()
    return nc


def _softmax_weights(nc, cpool, es_t, ed_t, cols, tagp):
    lg = cpool.tile([128, cols], F32, tag=f"lg{tagp}", name=f"lg{tagp}")
    nc.vector.tensor_tensor(
        out=lg[:], in0=es_t[:], in1=ed_t[:], op=mybir.AluOpType.add
    )
    lg2 = cpool.tile([128, cols], F32, tag=f"lg2{tagp}", name=f"lg2{tagp}")
    nc.vector.tensor_scalar_mul(out=lg2[:], in0=lg[:], scalar1=0.2)
    nc.vector.tensor_tensor(
        out=lg[:], in0=lg[:], in1=lg2[:], op=mybir.AluOpType.max
    )
    p = cpool.tile([128, cols], F32, tag=f"p{tagp}", name=f"p{tagp}")
    nc.scalar.activation(
        out=p[:], in_=lg[:], func=mybir.ActivationFunctionType.Exp
    )
    return p


def _build_agg(d, g, relu):
    """Layer-1 aggregation. Source rows are fetched with chunked int16
    dma_gather calls (table split at NSPLIT; one call per chunk per half,
    ~7.4ns/row Q7 emission vs ~11ns/row for per-tile INDIRECT1D), then
    scattered per dst window with one-hot PE matmuls.
    Output ho: [NSP, d] fp16 (normalized aggregate + bias (+relu))."""
    wt, wtA, wstart = g["wt"], g["wtA"], g["wstart"]
    chmeta, blk_of = g["chmeta"], g["blk_of"]
    T = int(sum(wt))
    ELEM = 384
    nc = bacc.Bacc(num_devices=NCORES)
    tabA = nc.dram_tensor("tabA", [NSPLIT, ELEM], F16, kind="ExternalInput").ap()
    tabB = nc.dram_tensor("tabB", [N - NSPLIT, ELEM], F16, kind="ExternalInput").ap()
    idxA = nc.dram_tensor("idxA", [128, g["scA"]], mybir.dt.int16,
                          kind="ExternalInput").ap()
    idxB = nc.dram_tensor("idxB", [128, g["scB"]], mybir.dt.int16,
                          kind="ExternalInput").ap()
    selftab = nc.dram_tensor("selftab", [NSP, d + 1], F16, kind="ExternalInput").ap()
    vrep = nc.dram_tensor("vrep", [128, 4 * d], F16, kind="ExternalInput").ap()
    oh = nc.dram_tensor("oh", [128, T, 128], F16, kind="ExternalInput").ap()
    dstf = nc.dram_tensor("dstf", [128, T], F32, kind="ExternalInput").ap()
    esx = nc.dram_tensor("esx", [128, T], F32, kind="ExternalInput").ap()
    edx = nc.dram_tensor("edx", [128, T], F32, kind="ExternalInput").ap()
    esself = nc.dram_tensor("esself", [128, W], F32, kind="ExternalInput").ap()
    edself = nc.dram_tensor("edself", [128, W], F32, kind="ExternalInput").ap()
    iota = nc.dram_tensor("iota", [128, 128], F32, kind="ExternalInput").ap()
    iotac = nc.dram_tensor("iotac", [128, 1], F32, kind="ExternalInput").ap()
    br = nc.dram_tensor("br", [128, d], F32, kind="ExternalInput").ap()
    cols = nc.dram_tensor("cols", [128, 4 * W], F32, kind="ExternalOutput").ap()

    with tile.TileContext(nc) as tc:
        with (
            tc.tile_pool(name="const", bufs=1) as cpool,
            tc.tile_pool(name="g", bufs=2) as gpool,
            tc.tile_pool(name="sf", bufs=4) as sfpool,
            tc.tile_pool(name="s", bufs=8) as spool,
            tc.tile_pool(name="obh", bufs=3) as ohpool,
            tc.tile_pool(name="o", bufs=3) as opool,
            tc.tile_pool(name="cl", bufs=6) as clpool,
            tc.tile_pool(name="ps", bufs=4, space="PSUM") as pspool,
        ):
            idxAs = cpool.tile([128, g["scA"]], mybir.dt.int16)
            nc.sync.dma_start(out=idxAs[:], in_=idxA[:])
            idxBs = cpool.tile([128, g["scB"]], mybir.dt.int16)
            nc.sync.dma_start(out=idxBs[:], in_=idxB[:])
            dsts = cpool.tile([128, T], F32)
            nc.sync.dma_start(out=dsts[:], in_=dstf[:])
            esxs = cpool.tile([128, T], F32)
            nc.sync.dma_start(out=esxs[:], in_=esx[:])
            edxs = cpool.tile([128, T], F32)
            nc.sync.dma_start(out=edxs[:], in_=edx[:])
            esss = cpool.tile([128, W], F32)
            nc.sync.dma_start(out=esss[:], in_=esself[:])
            edss = cpool.tile([128, W], F32)
            nc.sync.dma_start(out=edss[:], in_=edself[:])
            iosb = cpool.tile([128, 128], F32)
            nc.sync.dma_start(out=iosb[:], in_=iota[:])
            iocs = cpool.tile([128, 1], F32)
            nc.sync.dma_start(out=iocs[:], in_=iotac[:])
            brs = cpool.tile([128, d], F32)
            nc.sync.dma_start(out=brs[:], in_=br[:])
            vreps = cpool.tile([128, 4 * d], F16)
            nc.sync.dma_start(out=vreps[:], in_=vrep[:])
            colsb = cpool.tile([128, 4 * W], F32)

            p_all = _softmax_weights(nc, cpool, esxs, edxs, T, "e")
            p_self = _softmax_weights(nc, cpool, esss, edss, W, "s")

            caoff, cboff = 0, 0
            ohb, ohb_base = None, -10
            for wins, tA, tB in chmeta:
                nA, nB = len(tA), len(tB)
                gbuf = gpool.tile([128, nA + nB, ELEM], F16, tag="gb")
                if nA:
                    nc.gpsimd.dma_gather(
                        gbuf[:, 0:nA, :], tabA[:], idxAs[:, caoff:caoff + nA * 8],
                        nA * 128, nA * 128, ELEM, single_packet=False,
                    )
                    caoff += nA * 8
                if nB:
                    nc.gpsimd.dma_gather(
                        gbuf[:, nA:nA + nB, :], tabB[:],
                        idxBs[:, cboff:cboff + nB * 8],
                        nB * 128, nB * 128, ELEM, single_packet=False,
                    )
                    cboff += nB * 8
                for w in wins:
                    t = int(wstart[w])
                    ps = pspool.tile([128, d + 1], F32, space="PSUM")
                    st = sfpool.tile([128, d + 1], F16)
                    nc.sync.dma_start(
                        out=st[:], in_=selftab[128 * w:128 * (w + 1), :]
                    )
                    sd = spool.tile([128, 128], F16, tag="sdiag")
                    nc.vector.scalar_tensor_tensor(
                        out=sd[:], in0=iosb[:], scalar=iocs[:, :1],
                        in1=p_self[:, w:w + 1].to_broadcast([128, 128]),
                        op0=mybir.AluOpType.is_equal, op1=mybir.AluOpType.mult,
                    )
                    nc.tensor.matmul(
                        out=ps[:], lhsT=sd[:], rhs=st[:],
                        start=True, stop=(int(wt[w]) == 0),
                    )
                    for i in range(int(wt[w])):
                        blk = int(blk_of[t])
                        # one-hot lhsT on ACT from static 0/1 tiles so the
                        # Vector engine only runs epilogues (keeps the
                        # sp -> matmul -> buffer-free -> gather chain clear)
                        if ohb is None or t >= ohb_base + 8:
                            ohb = ohpool.tile([128, 8, 128], F16, tag="ohb")
                            nb = min(8, T - t)
                            nc.sync.dma_start(
                                out=ohb[:, 0:nb, :], in_=oh[:, t:t + nb, :]
                            )
                            ohb_base = t
                        sp = spool.tile([128, 128], F16, tag="sedge")
                        nc.scalar.mul(
                            out=sp[:], in_=ohb[:, t - ohb_base, :],
                            mul=p_all[:, t:t + 1],
                        )
                        nc.tensor.matmul(
                            out=ps[:], lhsT=sp[:], rhs=gbuf[:, blk, 0:d + 1],
                            start=False, stop=(i == int(wt[w]) - 1),
                        )
                        t += 1
                    _agg_epilogue(nc, clpool, opool, ps, brs, vreps, colsb, d, w)
            nc.sync.dma_start(out=cols[:], in_=colsb[:])
    nc.compile()
    return nc


def _agg_epilogue(nc, clpool, opool, ps, brs, vreps, colsb, d, w):
    """h1r = relu(agg/denom + b1); then the fused layer-2 projection columns
    es2/ed2/z0p/z1p = h1r . (W2@a_s2 | W2@a_d2 | W2@wl0 | W2@wl1)."""
    rec = clpool.tile([128, 1], F32)
    nc.vector.reciprocal(rec[:], ps[:, d:d + 1])
    ot = opool.tile([128, d], F32)
    nc.vector.scalar_tensor_tensor(
        out=ot[:], in0=ps[:, 0:d], scalar=rec[:, :1], in1=brs[:],
        op0=mybir.AluOpType.mult, op1=mybir.AluOpType.add,
    )
    ot16 = opool.tile([128, d], F16, tag="o16")
    nc.vector.tensor_scalar_max(out=ot16[:], in0=ot[:], scalar1=0.0)
    for j in range(4):
        scr = opool.tile([128, d], F16, tag=f"scr{j}")
        nc.vector.tensor_tensor(
            out=scr[:], in0=ot16[:], in1=vreps[:, d * j:d * (j + 1)],
            op=mybir.AluOpType.mult,
        )
        nc.vector.reduce_sum(
            out=colsb[:, 4 * w + j:4 * w + j + 1], in_=scr[:],
            axis=mybir.AxisListType.X,
        )


def _build_agg_z(wt):
    """Layer-2 scalar aggregation: gather-free. Per edge slot the rhs operands
    are host-expanded scalars [z0p[src], z1p[src], 1, 0]; the one-hot matmul
    scatters them by dst; epilogue: z_j[dst] = ps[:, j]/ps[:, 2] + c_j.
    One-hot lhsT tiles alternate between DVE is_eq builds and ACT-scaled
    static 0/1 tiles streamed from DRAM, halving the per-tile serial cost."""
    T = int(sum(wt))
    nc = bacc.Bacc(num_devices=NCORES)
    zin = nc.dram_tensor("zin", [128, T, 4], F16, kind="ExternalInput").ap()
    selfz = nc.dram_tensor("selfz", [128, W, 4], F16, kind="ExternalInput").ap()
    oh = nc.dram_tensor("oh", [128, T, 128], F16, kind="ExternalInput").ap()
    dstf = nc.dram_tensor("dstf", [128, T], F32, kind="ExternalInput").ap()
    esx = nc.dram_tensor("esx", [128, T], F32, kind="ExternalInput").ap()
    edx = nc.dram_tensor("edx", [128, T], F32, kind="ExternalInput").ap()
    esself = nc.dram_tensor("esself", [128, W], F32, kind="ExternalInput").ap()
    edself = nc.dram_tensor("edself", [128, W], F32, kind="ExternalInput").ap()
    iota = nc.dram_tensor("iota", [128, 128], F32, kind="ExternalInput").ap()
    iotac = nc.dram_tensor("iotac", [128, 1], F32, kind="ExternalInput").ap()
    c01 = nc.dram_tensor("c01", [128, 2], F32, kind="ExternalInput").ap()
    zo = nc.dram_tensor("zo", [128, 2 * W], F32, kind="ExternalOutput").ap()

    with tile.TileContext(nc) as tc:
        with (
            tc.tile_pool(name="const", bufs=1) as cpool,
            tc.tile_pool(name="s", bufs=8) as spool,
            tc.tile_pool(name="obh", bufs=3) as ohpool,
            tc.tile_pool(name="cl", bufs=6) as clpool,
            tc.tile_pool(name="ps", bufs=4, space="PSUM") as pspool,
        ):
            zins = cpool.tile([128, T, 4], F16)
            nc.sync.dma_start(out=zins[:], in_=zin[:])
            selfzs = cpool.tile([128, W, 4], F16)
            nc.sync.dma_start(out=selfzs[:], in_=selfz[:])
            dsts = cpool.tile([128, T], F32)
            nc.sync.dma_start(out=dsts[:], in_=dstf[:])
            esxs = cpool.tile([128, T], F32)
            nc.sync.dma_start(out=esxs[:], in_=esx[:])
            edxs = cpool.tile([128, T], F32)
            nc.sync.dma_start(out=edxs[:], in_=edx[:])
            esss = cpool.tile([128, W], F32)
            nc.sync.dma_start(out=esss[:], in_=esself[:])
            edss = cpool.tile([128, W], F32)
            nc.sync.dma_start(out=edss[:], in_=edself[:])
            iosb = cpool.tile([128, 128], F32)
            nc.sync.dma_start(out=iosb[:], in_=iota[:])
            iocs = cpool.tile([128, 1], F32)
            nc.sync.dma_start(out=iocs[:], in_=iotac[:])
            c01s = cpool.tile([128, 2], F32)
            nc.sync.dma_start(out=c01s[:], in_=c01[:])
            zob = cpool.tile([128, 2 * W], F32)

            p_all = _softmax_weights(nc, cpool, esxs, edxs, T, "e")
            p_self = _softmax_weights(nc, cpool, esss, edss, W, "s")

            t = 0
            ohb, ohb_base = None, -1
            for w in range(W):
                ps = pspool.tile([128, 4], F32, space="PSUM")
                sd = spool.tile([128, 128], F16, tag="sdiag")
                nc.vector.scalar_tensor_tensor(
                    out=sd[:], in0=iosb[:], scalar=iocs[:, :1],
                    in1=p_self[:, w:w + 1].to_broadcast([128, 128]),
                    op0=mybir.AluOpType.is_equal, op1=mybir.AluOpType.mult,
                )
                nc.tensor.matmul(
                    out=ps[:], lhsT=sd[:], rhs=selfzs[:, w, 0:4],
                    start=True, stop=(int(wt[w]) == 0),
                )
                for i in range(int(wt[w])):
                    if t % 3 == 0:
                        # ACT path: static 0/1 tile (batch-loaded) scaled by p
                        if ohb is None or t >= ohb_base + 8:
                            ohb = ohpool.tile([128, 8, 128], F16, tag="ohb")
                            nb = min(8, T - t)
                            nc.sync.dma_start(
                                out=ohb[:, 0:nb, :], in_=oh[:, t:t + nb, :]
                            )
                            ohb_base = t
                        sp = spool.tile([128, 128], F16, tag="sedge")
                        nc.scalar.mul(
                            out=sp[:], in_=ohb[:, t - ohb_base, :],
                            mul=p_all[:, t:t + 1],
                        )
                    else:
                        sp = spool.tile([128, 128], F16, tag="sedge")
                        nc.vector.scalar_tensor_tensor(
                            out=sp[:], in0=iosb[:], scalar=dsts[:, t:t + 1],
                            in1=p_all[:, t:t + 1].to_broadcast([128, 128]),
                            op0=mybir.AluOpType.is_equal, op1=mybir.AluOpType.mult,
                        )
                    nc.tensor.matmul(
                        out=ps[:], lhsT=sp[:], rhs=zins[:, t, 0:4],
                        start=False, stop=(i == int(wt[w]) - 1),
                    )
                    t += 1
                rec = clpool.tile([128, 1], F32)
                nc.vector.reciprocal(rec[:], ps[:, 2:3])
                nc.vector.scalar_tensor_tensor(
                    out=zob[:, 2 * w:2 * w + 2], in0=ps[:, 0:2],
                    scalar=rec[:, :1], in1=c01s[:, 0:2],
                    op0=mybir.AluOpType.mult, op1=mybir.AluOpType.add,
                )
            nc.sync.dma_start(out=zo[:], in_=zob[:])
    nc.compile()
    return nc


def _build_link2(pt):
    """z = sigmoid(z0x + z1x + bl) for pt*128 host-arranged pairs."""
    nc = bacc.Bacc(num_devices=NCORES)
    z0x = nc.dram_tensor("z0x", [128, pt], F32, kind="ExternalInput").ap()
    z1x = nc.dram_tensor("z1x", [128, pt], F32, kind="ExternalInput").ap()
    blr = nc.dram_tensor("blr", [128, 1], F32, kind="ExternalInput").ap()
    z = nc.dram_tensor("z", [128, pt], F32, kind="ExternalOutput").ap()

    with tile.TileContext(nc) as tc:
        with tc.tile_pool(name="c", bufs=1) as cpool:
            z0s = cpool.tile([128, pt], F32)
            nc.sync.dma_start(out=z0s[:], in_=z0x[:])
            z1s = cpool.tile([128, pt], F32)
            nc.sync.dma_start(out=z1s[:], in_=z1x[:])
            bls = cpool.tile([128, 1], F32)
            nc.sync.dma_start(out=bls[:], in_=blr[:])
            zs = cpool.tile([128, pt], F32, name="zs")
            nc.vector.tensor_tensor(
                out=zs[:], in0=z0s[:], in1=z1s[:], op=mybir.AluOpType.add
            )
            zsb = cpool.tile([128, pt], F32, name="zsb")
            nc.scalar.activation(
                out=zsb[:], in_=zs[:],
                func=mybir.ActivationFunctionType.Sigmoid, bias=bls[:, :1],
            )
            nc.sync.dma_start(out=z[:], in_=zsb[:])
    nc.compile()
    return nc


def _run(name, nc, in_maps, trace=True):
    last = None
    for attempt in range(3):
        try:
            res = run_bass_kernel_spmd(
                nc, in_maps, core_ids=list(range(NCORES)), trace=trace
            )
            LAST_EXEC_NS[name] = res.exec_time_ns
            return res.results
        except Exception as e:  # wedged-device retry (clears on re-attempt)
            last = e
            time.sleep(5)
    raise last


def _rep(v, n=128):
    return np.ascontiguousarray(np.broadcast_to(np.asarray(v, np.float32), (n, len(v))))


def _tile_xT(xfull_shards, kc, d_in):
    """list of [NSP, d_in] fp16 per core -> [NCORES, kc, W, 128, 128] fp16."""
    out = np.zeros((NCORES, kc, W, 128, 128), np.float16)
    for c in range(NCORES):
        xt = xfull_shards[c].T  # [d_in, NSP]
        for k in range(kc):
            blk = xt[128 * k:128 * (k + 1)].reshape(128, W, 128)
            out[c, k] = np.transpose(blk, (1, 0, 2))
    return out


# ------------------------------------------------------------------- kernel
def kernel(features, edge_index, mask, W1, a_src1, a_dst1, b1, W2, a_src2,
           a_dst2, b2, Wl, bl):
    features = np.asarray(features, np.float32)
    edge_index = np.asarray(edge_index, np.int32)
    mask = np.asarray(mask, np.int32)
    W1, W2, Wl = (np.asarray(a, np.float32) for a in (W1, W2, Wl))
    a_src1, a_dst1, b1 = (np.asarray(a, np.float32) for a in (a_src1, a_dst1, b1))
    a_src2, a_dst2, b2 = (np.asarray(a, np.float32) for a in (a_src2, a_dst2, b2))
    bl = np.asarray(bl, np.float32)
    wl0, wl1 = Wl[:F_IN, 0], Wl[F_IN:, 0]

    g = _prep_graph(edge_index)
    iota = np.ascontiguousarray(
        np.broadcast_to(np.arange(128, dtype=np.float32), (128, 128))
    )
    iotac = np.arange(128, dtype=np.float32).reshape(128, 1)

    key = (g["T"], tuple(int(x) for x in g["wt"]))
    if key not in _PROG_CACHE:
        _PROG_CACHE[key] = dict(
            p1=_build_proj(1, H, link_cols=False),
            a1=_build_agg(H, g, relu=True),
            az=_build_agg_z(g["wt"]),
            lk=_build_link2((10000 // NCORES + 127) // 128),
        )
    progs = _PROG_CACHE[key]

    # ---- L1: H1 = X @ W1 (sharded), es1/ed1
    xsh = []
    for c in range(NCORES):
        xs = np.zeros((NSP, F_IN), np.float16)
        xs[:NS] = features[c * NS:(c + 1) * NS]
        xsh.append(xs)
    xT1 = _tile_xT(xsh, 1, F_IN)
    W1h = W1.astype(np.float16)
    r1 = _run("p1", progs["p1"], [
        dict(xT=xT1[c], Wm=W1h, asr=_rep(a_src1), adr=_rep(a_dst1))
        for c in range(NCORES)
    ])
    H1e = np.concatenate([r1[c]["h16"][:NS] for c in range(NCORES)])   # [N, H+1] f16
    es1 = np.concatenate([r1[c]["cols"][:, 0::2].T.ravel()[:NS] for c in range(NCORES)])
    ed1 = np.concatenate([r1[c]["cols"][:, 1::2].T.ravel()[:NS] for c in range(NCORES)])

    # ---- L2: aggregate layer 1, then fused in-epilogue layer-2 projection:
    # cols = [es2 | ed2 | z0p | z1p] per local node (h1r never leaves device)
    b1r = _rep(b1)
    T1 = np.zeros((N, 384), np.float16)
    T1[:, 0:H + 1] = H1e
    vfold = np.stack([W2 @ a_src2, W2 @ a_dst2, W2 @ wl0, W2 @ wl1])   # [4, 256]
    vrep = np.ascontiguousarray(np.broadcast_to(
        vfold.reshape(1, 4 * H), (128, 4 * H))).astype(np.float16)
    ohs = []
    for c in range(NCORES):
        ohc = np.zeros((128, g["T"], 128), np.float16)
        pp, tt = np.nonzero(g["kind"][c] == 0)
        ohc[pp, tt, g["dstf"][c][pp, tt].astype(np.int64)] = 1.0
        ohs.append(ohc)
    ins2 = []
    for c in range(NCORES):
        esx, edx, ess, eds = _expand(es1, ed1, g, c)
        st = np.zeros((NSP, H + 1), np.float16)
        st[:NS] = H1e[c * NS:(c + 1) * NS]
        ins2.append(dict(tabA=T1[:NSPLIT], tabB=T1[NSPLIT:],
                         idxA=g["idxA"][c], idxB=g["idxB"][c],
                         selftab=st, vrep=vrep, oh=ohs[c], dstf=g["dstf"][c],
                         esx=esx, edx=edx, esself=ess, edself=eds,
                         iota=iota, iotac=iotac, br=b1r))
    r2 = _run("a1", progs["a1"], ins2)
    es2 = np.concatenate([r2[c]["cols"][:, 0::4].T.ravel()[:NS] for c in range(NCORES)])
    ed2 = np.concatenate([r2[c]["cols"][:, 1::4].T.ravel()[:NS] for c in range(NCORES)])
    z0p = np.concatenate([r2[c]["cols"][:, 2::4].T.ravel()[:NS] for c in range(NCORES)])
    z1p = np.concatenate([r2[c]["cols"][:, 3::4].T.ravel()[:NS] for c in range(NCORES)])

    # ---- L4: scalar aggregation -> z0/z1 per node
    c0 = float(b2 @ wl0)
    c1 = float(b2 @ wl1)
    c01 = np.ascontiguousarray(
        np.broadcast_to(np.array([c0, c1], np.float32), (128, 2))
    )
    ins4 = []
    for c in range(NCORES):
        esx, edx, ess, eds = _expand(es2, ed2, g, c)
        real = (g["kind"][c] == 0)
        zin = np.zeros((128, g["T"], 4), np.float16)
        zin[:, :, 0] = np.where(real, z0p[g["srcs"][c]], 0.0)
        zin[:, :, 1] = np.where(real, z1p[g["srcs"][c]], 0.0)
        zin[:, :, 2] = real.astype(np.float16)
        nid = np.arange(NSP)
        nglob = np.minimum(c * NS + nid, N - 1)
        valid = (nid < NS)
        selfz = np.zeros((128, W, 4), np.float16)
        selfz[:, :, 0] = np.where(valid, z0p[nglob], 0.0).reshape(W, 128).T
        selfz[:, :, 1] = np.where(valid, z1p[nglob], 0.0).reshape(W, 128).T
        selfz[:, :, 2] = valid.astype(np.float16).reshape(W, 128).T
        ins4.append(dict(zin=zin, selfz=selfz, oh=ohs[c], dstf=g["dstf"][c],
                         esx=esx, edx=edx, esself=ess, edself=eds,
                         iota=iota, iotac=iotac, c01=c01))
    r4 = _run("az", progs["az"], ins4)
    zoc = [r4[c]["zo"] for c in range(NCORES)]         # [128, 2W] f32 per core
    z0f = np.concatenate(
        [zoc[c][:, 0::2].T.ravel()[:NS] for c in range(NCORES)])
    z1f = np.concatenate(
        [zoc[c][:, 1::2].T.ravel()[:NS] for c in range(NCORES)])

    # ---- L5: z = sigmoid(z0[m0] + z1[m1] + bl)
    P = mask.shape[0]
    pc = P // NCORES
    pt = (pc + 127) // 128
    z0x = np.zeros((NCORES, 128, pt), np.float32)
    z1x = np.zeros((NCORES, 128, pt), np.float32)
    mT = mask.T
    for c in range(NCORES):
        s = np.arange(pc)
        z0x[c, s % 128, s // 128] = z0f[mT[0][c * pc:(c + 1) * pc]]
        z1x[c, s % 128, s // 128] = z1f[mT[1][c * pc:(c + 1) * pc]]
    blr = np.full((128, 1), float(bl[0]), np.float32)
    r5 = _run("lk", progs["lk"], [
        dict(z0x=z0x[c], z1x=z1x[c], blr=blr)
        for c in range(NCORES)
    ])
    out = np.zeros((P, 1), np.float32)
    for c in range(NCORES):
        s = np.arange(pc)
        out[c * pc:(c + 1) * pc, 0] = r5[c]["z"][s % 128, s // 128]

    tot = sum(v for v in LAST_EXEC_NS.values() if v)
    print(f"kernel launches ns: {LAST_EXEC_NS} total {tot}")
    return out


# revision 36
# speedup vs baseline: 1.1342x; 1.0500x over previous
"""Two-layer GAT (single-head, PyG-style) + link predictor on 8 TRN2 NeuronCores.

v2 strategy (memory-regime). Key observation: the second GAT layer's output h2
is consumed ONLY through four linear functionals per node — es2 = h2p@a_s2,
ed2 = h2p@a_d2, z0 = h2@wl0, z1 = h2@wl1 (wl0/wl1 = halves of the link weight
column). By linearity of the edge-softmax aggregation, layer-2 aggregation
therefore reduces to aggregating per-edge SCALARS:
    z0[dst] = (sum_e alpha_e * z0p[src_e]) / denom + b2.wl0
so launch 4 needs NO feature gather at all: the per-edge scalar operands
(z0p[src], z1p[src], 1) are host-expanded index-space arrays (same class as
the esx/edx expansions the v1 kernel already used), and the one-hot matmul
scatters them per destination window with a 4-wide rhs.

Launches:
  L1 p1 : h1p = X@W1 (+ folded es1/ed1 cols)        [unchanged from v1]
  L2 a1 : edge-softmax aggregate of h1p rows via per-tile indirect row
          gathers + one-hot PE matmuls; h1r = relu(agg+b1)   [unchanged]
  L3 p2 : h2p-projection folds extended to SIX columns:
          [W2 | W2@a_s2 | W2@a_d2 | W2@wl0 | W2@wl1]; outputs only the four
          per-node scalars (h2p itself never materializes in DRAM).
  L4 az : scalar aggregation (rhs = [z0x, z1x, 1, 0] per edge slot) ->
          z0/z1 per node. No gathers, PE cost (128+4) cols per tile.
  L5 lk : z = sigmoid(z0[m0] + z1[m1] + bl) on host-arranged index pairs.
"""
import time

import numpy as np

import concourse.bass as bass
import concourse.mybir as mybir
import concourse.tile as tile
from concourse import bacc
from concourse.bass_utils import run_bass_kernel_spmd

F32 = mybir.dt.float32
F16 = mybir.dt.float16
I32 = mybir.dt.int32

NCORES = 8
N, F_IN, H, C = 50000, 128, 256, 1
NS = N // NCORES            # 6250 nodes per shard
W = (NS + 127) // 128       # 49 windows per shard
NSP = W * 128               # 6272 padded slots
NEG = -1.0e30               # pad-edge sentinel (exp -> exactly 0)

LAST_EXEC_NS = {}           # launch name -> exec_time_ns (filled per kernel() call)
_PROG_CACHE = {}


# ----------------------------------------------------------------- host prep
NSPLIT = 32768          # int16 table split point for dma_gather
CH_TILES = 60           # max tiles per gather chunk (SBUF-bound, 3 bufs)


def _prep_graph(edge_index):
    """Partition non-self edges by dst shard; within each dst window, edges
    are segregated by src half (A: src<32768, B: src>=32768) into separate
    128-slot tiles so a chunked int16 dma_gather can fetch whole tile groups.
    Common (max-over-cores) per-window tile counts wtA/wtB keep the program
    SPMD-shared. Edge slot s of a window region is (t, p) = (s//128, s%128)."""
    src = np.asarray(edge_index[0], np.int64)
    dst = np.asarray(edge_index[1], np.int64)

    core = dst // NS
    order = np.argsort(dst, kind="stable")
    src, dst, core = src[order], dst[order], core[order]

    e_src, e_dstloc = [], []
    for c in range(NCORES):
        m = core == c
        e_src.append(src[m])
        e_dstloc.append(dst[m] - c * NS)

    wtA = np.zeros(W, dtype=np.int64)
    wtB = np.zeros(W, dtype=np.int64)
    for c in range(NCORES):
        win = e_dstloc[c] // 128
        isA = e_src[c] < NSPLIT
        cntA = np.bincount(win[isA], minlength=W)
        cntB = np.bincount(win[~isA], minlength=W)
        wtA = np.maximum(wtA, (cntA + 127) // 128)
        wtB = np.maximum(wtB, (cntB + 127) // 128)
    wt = wtA + wtB
    T = int(wt.sum())
    wstart = np.concatenate([[0], np.cumsum(wt)]).astype(np.int64)

    srcs = np.zeros((NCORES, 128, T), dtype=np.int32)
    dstg = np.zeros((NCORES, 128, T), dtype=np.int32)
    dstf = np.full((NCORES, 128, T), -1.0, dtype=np.float32)
    kind = np.ones((NCORES, 128, T), dtype=np.int8)      # 0 real 1 pad
    # default pad sources: A tiles -> row 0, B tiles -> row NSPLIT
    for w in range(W):
        t0 = int(wstart[w])
        srcs[:, :, t0 + int(wtA[w]):t0 + int(wt[w])] = NSPLIT

    for c in range(NCORES):
        win = e_dstloc[c] // 128
        isA = e_src[c] < NSPLIT
        for w in range(W):
            t0 = int(wstart[w])
            for half, toff, flag in ((0, t0, True), (1, t0 + int(wtA[w]), False)):
                m = (win == w) & (isA == flag)
                s = e_src[c][m]
                dl = e_dstloc[c][m]
                n_e = len(s)
                sl = np.arange(n_e)
                tt, pp = toff + sl // 128, sl % 128
                srcs[c, pp, tt] = s
                dstg[c, pp, tt] = (dl + c * NS).astype(np.int32)
                dstf[c, pp, tt] = (dl - 128 * w).astype(np.float32)
                kind[c, pp, tt] = 0

    # chunk consecutive windows for the gather calls
    chunks = []
    cur = []
    cur_tiles = 0
    for w in range(W):
        tw = int(wt[w])
        if cur and cur_tiles + tw > CH_TILES:
            chunks.append(cur)
            cur, cur_tiles = [], 0
        cur.append(w)
        cur_tiles += tw
    if cur:
        chunks.append(cur)

    # per-chunk tile lists (global tile ids) for A and B calls + per-tile blk
    chmeta = []          # (wins, tilesA, tilesB) ; blk of tile = position
    blk_of = np.zeros(T, dtype=np.int64)
    for wins in chunks:
        tA, tB = [], []
        for w in wins:
            t0 = int(wstart[w])
            tA.extend(range(t0, t0 + int(wtA[w])))
            tB.extend(range(t0 + int(wtA[w]), t0 + int(wt[w])))
        for i, t in enumerate(tA):
            blk_of[t] = i
        for i, t in enumerate(tB):
            blk_of[t] = len(tA) + i
        chmeta.append((wins, tA, tB))

    # host-built int16 gather index arrays (16-partition wrap, replicated x8)
    def wrap16(vals):
        nidx = len(vals)
        colsn = (nidx + 15) // 16
        arr = np.zeros((128, colsn), np.int16)
        s = np.arange(nidx)
        for grp in range(8):
            arr[grp * 16 + s % 16, s // 16] = vals
        return arr

    idxA, idxB = [], []
    for c in range(NCORES):
        pa, pb = [], []
        for wins, tA, tB in chmeta:
            if tA:
                pa.append(wrap16(srcs[c][:, tA].T.ravel()))
            if tB:
                pb.append(wrap16((srcs[c][:, tB].T.ravel() - NSPLIT)))
        idxA.append(np.concatenate(pa, axis=1) if pa else np.zeros((128, 1), np.int16))
        idxB.append(np.concatenate(pb, axis=1) if pb else np.zeros((128, 1), np.int16))

    return dict(srcs=srcs, dstg=dstg, dstf=dstf, kind=kind, wt=wt,
                wtA=wtA, wtB=wtB, T=T, wstart=wstart, chmeta=chmeta,
                blk_of=blk_of, idxA=idxA, idxB=idxB,
                scA=idxA[0].shape[1], scB=idxB[0].shape[1])


def _expand(es_full, ed_full, g, c):
    """Host halo expansion: per-edge es[src], ed[dst] (+sentinel for pads),
    and per-node self-loop es/ed in [128, W] layout."""
    esx = es_full[g["srcs"][c]].astype(np.float32)
    edx = ed_full[np.minimum(g["dstg"][c], N - 1)].astype(np.float32)
    pad = g["kind"][c] == 1
    esx[pad] = NEG
    edx[pad] = 0.0
    nid = np.arange(NSP)
    nglob = np.minimum(c * NS + nid, N - 1)
    ess = np.where(nid < NS, es_full[nglob], 0.0).astype(np.float32)
    eds = np.where(nid < NS, ed_full[nglob], 0.0).astype(np.float32)
    return esx, edx, ess.reshape(W, 128).T.copy(), eds.reshape(W, 128).T.copy()


# ------------------------------------------------------------- bass programs
def _build_proj(kc, d_out, link_cols):
    """Projection per 128-node window: psum = x @ [W | W@a_s | W@a_d (|W@wl0
    |W@wl1)].  Inputs: xT fp16 [kc, W, 128, 128], Wm fp16 [kc*128, d_out],
    asr/adr (w0r/w1r) fp32 [128, d_out].
    Outputs: h16 [NSP, d_out+1] fp16 (feat + 1.0 col; omitted if link_cols),
    es/ed (z0/z1) [128, W] f32."""
    nf = 4 if link_cols else 2
    nc = bacc.Bacc(num_devices=NCORES)
    xT = nc.dram_tensor("xT", [kc, W, 128, 128], F16, kind="ExternalInput").ap()
    Wm = nc.dram_tensor("Wm", [kc * 128, d_out], F16, kind="ExternalInput").ap()
    folds = []
    for j, nm in enumerate(["asr", "adr", "w0r", "w1r"][:nf]):
        folds.append(nc.dram_tensor(nm, [128, d_out], F32, kind="ExternalInput").ap())
    if not link_cols:
        h16 = nc.dram_tensor("h16", [NSP, d_out + 1], F16, kind="ExternalOutput").ap()
    cols = nc.dram_tensor("cols", [128, nf * W], F32, kind="ExternalOutput").ap()

    with tile.TileContext(nc) as tc:
        with (
            tc.tile_pool(name="const", bufs=1) as cpool,
            tc.tile_pool(name="x", bufs=6) as xpool,
            tc.tile_pool(name="o", bufs=4) as opool,
            tc.tile_pool(name="ps", bufs=4, space="PSUM") as pspool,
            tc.tile_pool(name="sc", bufs=4) as scpool,
        ):
            fsb = []
            for j in range(nf):
                fb = cpool.tile([128, d_out], F32, tag=f"f{j}")
                nc.sync.dma_start(out=fb[:], in_=folds[j])
                fsb.append(fb)
            obuf = cpool.tile([128, nf * W], F32, name="obuf")

            wsb = []
            for k in range(kc):
                wk = cpool.tile([128, d_out + nf], F16, tag=f"w{k}")
                nc.sync.dma_start(
                    out=wk[:, 0:d_out], in_=Wm[128 * k:128 * (k + 1), :]
                )
                for j in range(nf):
                    scr = scpool.tile([128, d_out], F32, tag=f"wf{j}")
                    nc.vector.tensor_tensor(
                        out=scr[:], in0=wk[:, 0:d_out], in1=fsb[j][:],
                        op=mybir.AluOpType.mult,
                    )
                    wcol = scpool.tile([128, 1], F32, tag=f"wc{j}")
                    nc.vector.reduce_sum(
                        out=wcol[:], in_=scr[:], axis=mybir.AxisListType.X
                    )
                    nc.vector.tensor_copy(
                        out=wk[:, d_out + j:d_out + j + 1], in_=wcol[:]
                    )
                wsb.append(wk)

            for w in range(W):
                ps = pspool.tile([128, d_out + nf], F32, space="PSUM")
                for k in range(kc):
                    xt = xpool.tile([128, 128], F16)
                    nc.sync.dma_start(out=xt[:], in_=xT[k, w])
                    nc.tensor.matmul(
                        out=ps[:], lhsT=xt[:], rhs=wsb[k][:],
                        start=(k == 0), stop=(k == kc - 1),
                    )
                if not link_cols:
                    ht = opool.tile([128, d_out + 1], F16)
                    nc.scalar.copy(out=ht[:, 0:d_out], in_=ps[:, 0:d_out])
                    nc.vector.memset(ht[:, d_out:d_out + 1], 1.0)
                    nc.sync.dma_start(out=h16[128 * w:128 * (w + 1), :], in_=ht[:])
                nc.vector.tensor_copy(
                    out=obuf[:, nf * w:nf * (w + 1)],
                    in_=ps[:, d_out:d_out + nf],
                )
            nc.sync.dma_start(out=cols[:], in_=obuf[:])
    nc.compile()
    return nc


def _softmax_weights(nc, cpool, es_t, ed_t, cols, tagp):
    lg = cpool.tile([128, cols], F32, tag=f"lg{tagp}", name=f"lg{tagp}")
    nc.vector.tensor_tensor(
        out=lg[:], in0=es_t[:], in1=ed_t[:], op=mybir.AluOpType.add
    )
    lg2 = cpool.tile([128, cols], F32, tag=f"lg2{tagp}", name=f"lg2{tagp}")
    nc.vector.tensor_scalar_mul(out=lg2[:], in0=lg[:], scalar1=0.2)
    nc.vector.tensor_tensor(
        out=lg[:], in0=lg[:], in1=lg2[:], op=mybir.AluOpType.max
    )
    p = cpool.tile([128, cols], F32, tag=f"p{tagp}", name=f"p{tagp}")
    nc.scalar.activation(
        out=p[:], in_=lg[:], func=mybir.ActivationFunctionType.Exp
    )
    return p


def _build_agg(d, g, relu):
    """Layer-1 aggregation. Source rows are fetched with chunked int16
    dma_gather calls (table split at NSPLIT; one call per chunk per half,
    ~7.4ns/row Q7 emission vs ~11ns/row for per-tile INDIRECT1D), then
    scattered per dst window with one-hot PE matmuls.
    Output ho: [NSP, d] fp16 (normalized aggregate + bias (+relu))."""
    wt, wtA, wstart = g["wt"], g["wtA"], g["wstart"]
    chmeta, blk_of = g["chmeta"], g["blk_of"]
    T = int(sum(wt))
    ELEM = 384
    nc = bacc.Bacc(num_devices=NCORES)
    tabA = nc.dram_tensor("tabA", [NSPLIT, ELEM], F16, kind="ExternalInput").ap()
    tabB = nc.dram_tensor("tabB", [N - NSPLIT, ELEM], F16, kind="ExternalInput").ap()
    idxA = nc.dram_tensor("idxA", [128, g["scA"]], mybir.dt.int16,
                          kind="ExternalInput").ap()
    idxB = nc.dram_tensor("idxB", [128, g["scB"]], mybir.dt.int16,
                          kind="ExternalInput").ap()
    selftab = nc.dram_tensor("selftab", [NSP, d + 1], F16, kind="ExternalInput").ap()
    vrep = nc.dram_tensor("vrep", [128, 4 * d], F16, kind="ExternalInput").ap()
    oh = nc.dram_tensor("oh", [128, T, 128], F16, kind="ExternalInput").ap()
    dstf = nc.dram_tensor("dstf", [128, T], F32, kind="ExternalInput").ap()
    esx = nc.dram_tensor("esx", [128, T], F32, kind="ExternalInput").ap()
    edx = nc.dram_tensor("edx", [128, T], F32, kind="ExternalInput").ap()
    esself = nc.dram_tensor("esself", [128, W], F32, kind="ExternalInput").ap()
    edself = nc.dram_tensor("edself", [128, W], F32, kind="ExternalInput").ap()
    iota = nc.dram_tensor("iota", [128, 128], F32, kind="ExternalInput").ap()
    iotac = nc.dram_tensor("iotac", [128, 1], F32, kind="ExternalInput").ap()
    br = nc.dram_tensor("br", [128, d], F32, kind="ExternalInput").ap()
    cols = nc.dram_tensor("cols", [128, 4 * W], F32, kind="ExternalOutput").ap()

    with tile.TileContext(nc) as tc:
        with (
            tc.tile_pool(name="const", bufs=1) as cpool,
            tc.tile_pool(name="g", bufs=3) as gpool,
            tc.tile_pool(name="sf", bufs=4) as sfpool,
            tc.tile_pool(name="s", bufs=8) as spool,
            tc.tile_pool(name="obh", bufs=3) as ohpool,
            tc.tile_pool(name="o", bufs=3) as opool,
            tc.tile_pool(name="cl", bufs=6) as clpool,
            tc.tile_pool(name="ps", bufs=4, space="PSUM") as pspool,
        ):
            idxAs = cpool.tile([128, g["scA"]], mybir.dt.int16)
            nc.sync.dma_start(out=idxAs[:], in_=idxA[:])
            idxBs = cpool.tile([128, g["scB"]], mybir.dt.int16)
            nc.sync.dma_start(out=idxBs[:], in_=idxB[:])
            dsts = cpool.tile([128, T], F32)
            nc.sync.dma_start(out=dsts[:], in_=dstf[:])
            esxs = cpool.tile([128, T], F32)
            nc.sync.dma_start(out=esxs[:], in_=esx[:])
            edxs = cpool.tile([128, T], F32)
            nc.sync.dma_start(out=edxs[:], in_=edx[:])
            esss = cpool.tile([128, W], F32)
            nc.sync.dma_start(out=esss[:], in_=esself[:])
            edss = cpool.tile([128, W], F32)
            nc.sync.dma_start(out=edss[:], in_=edself[:])
            iosb = cpool.tile([128, 128], F32)
            nc.sync.dma_start(out=iosb[:], in_=iota[:])
            iocs = cpool.tile([128, 1], F32)
            nc.sync.dma_start(out=iocs[:], in_=iotac[:])
            brs = cpool.tile([128, d], F32)
            nc.sync.dma_start(out=brs[:], in_=br[:])
            vreps = cpool.tile([128, 4 * d], F16)
            nc.sync.dma_start(out=vreps[:], in_=vrep[:])
            colsb = cpool.tile([128, 4 * W], F32)

            p_all = _softmax_weights(nc, cpool, esxs, edxs, T, "e")
            p_self = _softmax_weights(nc, cpool, esss, edss, W, "s")

            caoff, cboff = 0, 0
            ohb, ohb_base = None, -10
            for wins, tA, tB in chmeta:
                nA, nB = len(tA), len(tB)
                gbuf = gpool.tile([128, nA + nB, ELEM], F16, tag="gb")
                if nA:
                    nc.gpsimd.dma_gather(
                        gbuf[:, 0:nA, :], tabA[:], idxAs[:, caoff:caoff + nA * 8],
                        nA * 128, nA * 128, ELEM, single_packet=False,
                    )
                    caoff += nA * 8
                if nB:
                    nc.gpsimd.dma_gather(
                        gbuf[:, nA:nA + nB, :], tabB[:],
                        idxBs[:, cboff:cboff + nB * 8],
                        nB * 128, nB * 128, ELEM, single_packet=False,
                    )
                    cboff += nB * 8
                for w in wins:
                    t = int(wstart[w])
                    ps = pspool.tile([128, d + 1], F32, space="PSUM")
                    st = sfpool.tile([128, d + 1], F16)
                    nc.sync.dma_start(
                        out=st[:], in_=selftab[128 * w:128 * (w + 1), :]
                    )
                    sd = spool.tile([128, 128], F16, tag="sdiag")
                    nc.vector.scalar_tensor_tensor(
                        out=sd[:], in0=iosb[:], scalar=iocs[:, :1],
                        in1=p_self[:, w:w + 1].to_broadcast([128, 128]),
                        op0=mybir.AluOpType.is_equal, op1=mybir.AluOpType.mult,
                    )
                    nc.tensor.matmul(
                        out=ps[:], lhsT=sd[:], rhs=st[:],
                        start=True, stop=(int(wt[w]) == 0),
                    )
                    for i in range(int(wt[w])):
                        blk = int(blk_of[t])
                        # one-hot lhsT on ACT from static 0/1 tiles so the
                        # Vector engine only runs epilogues (keeps the
                        # sp -> matmul -> buffer-free -> gather chain clear)
                        if ohb is None or t >= ohb_base + 8:
                            ohb = ohpool.tile([128, 8, 128], F16, tag="ohb")
                            nb = min(8, T - t)
                            nc.sync.dma_start(
                                out=ohb[:, 0:nb, :], in_=oh[:, t:t + nb, :]
                            )
                            ohb_base = t
                        sp = spool.tile([128, 128], F16, tag="sedge")
                        nc.scalar.mul(
                            out=sp[:], in_=ohb[:, t - ohb_base, :],
                            mul=p_all[:, t:t + 1],
                        )
                        nc.tensor.matmul(
                            out=ps[:], lhsT=sp[:], rhs=gbuf[:, blk, 0:d + 1],
                            start=False, stop=(i == int(wt[w]) - 1),
                        )
                        t += 1
                    _agg_epilogue(nc, clpool, opool, ps, brs, vreps, colsb, d, w)
            nc.sync.dma_start(out=cols[:], in_=colsb[:])
    nc.compile()
    return nc


def _agg_epilogue(nc, clpool, opool, ps, brs, vreps, colsb, d, w):
    """h1r = relu(agg/denom + b1); then the fused layer-2 projection columns
    es2/ed2/z0p/z1p = h1r . (W2@a_s2 | W2@a_d2 | W2@wl0 | W2@wl1)."""
    rec = clpool.tile([128, 1], F32)
    nc.vector.reciprocal(rec[:], ps[:, d:d + 1])
    ot = opool.tile([128, d], F32)
    nc.vector.scalar_tensor_tensor(
        out=ot[:], in0=ps[:, 0:d], scalar=rec[:, :1], in1=brs[:],
        op0=mybir.AluOpType.mult, op1=mybir.AluOpType.add,
    )
    ot16 = opool.tile([128, d], F16, tag="o16")
    nc.vector.tensor_scalar_max(out=ot16[:], in0=ot[:], scalar1=0.0)
    for j in range(4):
        scr = opool.tile([128, d], F16, tag=f"scr{j}")
        nc.vector.tensor_tensor(
            out=scr[:], in0=ot16[:], in1=vreps[:, d * j:d * (j + 1)],
            op=mybir.AluOpType.mult,
        )
        nc.vector.reduce_sum(
            out=colsb[:, 4 * w + j:4 * w + j + 1], in_=scr[:],
            axis=mybir.AxisListType.X,
        )


def _build_agg_z(wt):
    """Layer-2 scalar aggregation: gather-free. Per edge slot the rhs operands
    are host-expanded scalars [z0p[src], z1p[src], 1, 0]; the one-hot matmul
    scatters them by dst; epilogue: z_j[dst] = ps[:, j]/ps[:, 2] + c_j.
    One-hot lhsT tiles alternate between DVE is_eq builds and ACT-scaled
    static 0/1 tiles streamed from DRAM, halving the per-tile serial cost."""
    T = int(sum(wt))
    nc = bacc.Bacc(num_devices=NCORES)
    zin = nc.dram_tensor("zin", [128, T, 4], F16, kind="ExternalInput").ap()
    selfz = nc.dram_tensor("selfz", [128, W, 4], F16, kind="ExternalInput").ap()
    oh = nc.dram_tensor("oh", [128, T, 128], F16, kind="ExternalInput").ap()
    dstf = nc.dram_tensor("dstf", [128, T], F32, kind="ExternalInput").ap()
    esx = nc.dram_tensor("esx", [128, T], F32, kind="ExternalInput").ap()
    edx = nc.dram_tensor("edx", [128, T], F32, kind="ExternalInput").ap()
    esself = nc.dram_tensor("esself", [128, W], F32, kind="ExternalInput").ap()
    edself = nc.dram_tensor("edself", [128, W], F32, kind="ExternalInput").ap()
    iota = nc.dram_tensor("iota", [128, 128], F32, kind="ExternalInput").ap()
    iotac = nc.dram_tensor("iotac", [128, 1], F32, kind="ExternalInput").ap()
    c01 = nc.dram_tensor("c01", [128, 2], F32, kind="ExternalInput").ap()
    zo = nc.dram_tensor("zo", [128, 2 * W], F32, kind="ExternalOutput").ap()

    with tile.TileContext(nc) as tc:
        with (
            tc.tile_pool(name="const", bufs=1) as cpool,
            tc.tile_pool(name="s", bufs=8) as spool,
            tc.tile_pool(name="obh", bufs=3) as ohpool,
            tc.tile_pool(name="cl", bufs=6) as clpool,
            tc.tile_pool(name="ps", bufs=4, space="PSUM") as pspool,
        ):
            zins = cpool.tile([128, T, 4], F16)
            nc.sync.dma_start(out=zins[:], in_=zin[:])
            selfzs = cpool.tile([128, W, 4], F16)
            nc.sync.dma_start(out=selfzs[:], in_=selfz[:])
            dsts = cpool.tile([128, T], F32)
            nc.sync.dma_start(out=dsts[:], in_=dstf[:])
            esxs = cpool.tile([128, T], F32)
            nc.sync.dma_start(out=esxs[:], in_=esx[:])
            edxs = cpool.tile([128, T], F32)
            nc.sync.dma_start(out=edxs[:], in_=edx[:])
            esss = cpool.tile([128, W], F32)
            nc.sync.dma_start(out=esss[:], in_=esself[:])
            edss = cpool.tile([128, W], F32)
            nc.sync.dma_start(out=edss[:], in_=edself[:])
            iosb = cpool.tile([128, 128], F32)
            nc.sync.dma_start(out=iosb[:], in_=iota[:])
            iocs = cpool.tile([128, 1], F32)
            nc.sync.dma_start(out=iocs[:], in_=iotac[:])
            c01s = cpool.tile([128, 2], F32)
            nc.sync.dma_start(out=c01s[:], in_=c01[:])
            zob = cpool.tile([128, 2 * W], F32)

            p_all = _softmax_weights(nc, cpool, esxs, edxs, T, "e")
            p_self = _softmax_weights(nc, cpool, esss, edss, W, "s")

            t = 0
            ohb, ohb_base = None, -1
            for w in range(W):
                ps = pspool.tile([128, 4], F32, space="PSUM")
                sd = spool.tile([128, 128], F16, tag="sdiag")
                nc.vector.scalar_tensor_tensor(
                    out=sd[:], in0=iosb[:], scalar=iocs[:, :1],
                    in1=p_self[:, w:w + 1].to_broadcast([128, 128]),
                    op0=mybir.AluOpType.is_equal, op1=mybir.AluOpType.mult,
                )
                nc.tensor.matmul(
                    out=ps[:], lhsT=sd[:], rhs=selfzs[:, w, 0:4],
                    start=True, stop=(int(wt[w]) == 0),
                )
                for i in range(int(wt[w])):
                    if t % 3 == 0:
                        # ACT path: static 0/1 tile (batch-loaded) scaled by p
                        if ohb is None or t >= ohb_base + 8:
                            ohb = ohpool.tile([128, 8, 128], F16, tag="ohb")
                            nb = min(8, T - t)
                            nc.sync.dma_start(
                                out=ohb[:, 0:nb, :], in_=oh[:, t:t + nb, :]
                            )
                            ohb_base = t
                        sp = spool.tile([128, 128], F16, tag="sedge")
                        nc.scalar.mul(
                            out=sp[:], in_=ohb[:, t - ohb_base, :],
                            mul=p_all[:, t:t + 1],
                        )
                    else:
                        sp = spool.tile([128, 128], F16, tag="sedge")
                        nc.vector.scalar_tensor_tensor(
                            out=sp[:], in0=iosb[:], scalar=dsts[:, t:t + 1],
                            in1=p_all[:, t:t + 1].to_broadcast([128, 128]),
                            op0=mybir.AluOpType.is_equal, op1=mybir.AluOpType.mult,
                        )
                    nc.tensor.matmul(
                        out=ps[:], lhsT=sp[:], rhs=zins[:, t, 0:4],
                        start=False, stop=(i == int(wt[w]) - 1),
                    )
                    t += 1
                rec = clpool.tile([128, 1], F32)
                nc.vector.reciprocal(rec[:], ps[:, 2:3])
                nc.vector.scalar_tensor_tensor(
                    out=zob[:, 2 * w:2 * w + 2], in0=ps[:, 0:2],
                    scalar=rec[:, :1], in1=c01s[:, 0:2],
                    op0=mybir.AluOpType.mult, op1=mybir.AluOpType.add,
                )
            nc.sync.dma_start(out=zo[:], in_=zob[:])
    nc.compile()
    return nc


def _build_link2(pt):
    """z = sigmoid(z0x + z1x + bl) for pt*128 host-arranged pairs."""
    nc = bacc.Bacc(num_devices=NCORES)
    z0x = nc.dram_tensor("z0x", [128, pt], F32, kind="ExternalInput").ap()
    z1x = nc.dram_tensor("z1x", [128, pt], F32, kind="ExternalInput").ap()
    blr = nc.dram_tensor("blr", [128, 1], F32, kind="ExternalInput").ap()
    z = nc.dram_tensor("z", [128, pt], F32, kind="ExternalOutput").ap()

    with tile.TileContext(nc) as tc:
        with tc.tile_pool(name="c", bufs=1) as cpool:
            z0s = cpool.tile([128, pt], F32)
            nc.sync.dma_start(out=z0s[:], in_=z0x[:])
            z1s = cpool.tile([128, pt], F32)
            nc.sync.dma_start(out=z1s[:], in_=z1x[:])
            bls = cpool.tile([128, 1], F32)
            nc.sync.dma_start(out=bls[:], in_=blr[:])
            zs = cpool.tile([128, pt], F32, name="zs")
            nc.vector.tensor_tensor(
                out=zs[:], in0=z0s[:], in1=z1s[:], op=mybir.AluOpType.add
            )
            zsb = cpool.tile([128, pt], F32, name="zsb")
            nc.scalar.activation(
                out=zsb[:], in_=zs[:],
                func=mybir.ActivationFunctionType.Sigmoid, bias=bls[:, :1],
            )
            nc.sync.dma_start(out=z[:], in_=zsb[:])
    nc.compile()
    return nc


def _run(name, nc, in_maps, trace=True):
    last = None
    for attempt in range(3):
        try:
            res = run_bass_kernel_spmd(
                nc, in_maps, core_ids=list(range(NCORES)), trace=trace
            )
            LAST_EXEC_NS[name] = res.exec_time_ns
            return res.results
        except Exception as e:  # wedged-device retry (clears on re-attempt)
            last = e
            time.sleep(5)
    raise last


def _rep(v, n=128):
    return np.ascontiguousarray(np.broadcast_to(np.asarray(v, np.float32), (n, len(v))))


def _tile_xT(xfull_shards, kc, d_in):
    """list of [NSP, d_in] fp16 per core -> [NCORES, kc, W, 128, 128] fp16."""
    out = np.zeros((NCORES, kc, W, 128, 128), np.float16)
    for c in range(NCORES):
        xt = xfull_shards[c].T  # [d_in, NSP]
        for k in range(kc):
            blk = xt[128 * k:128 * (k + 1)].reshape(128, W, 128)
            out[c, k] = np.transpose(blk, (1, 0, 2))
    return out


# ------------------------------------------------------------------- kernel
def kernel(features, edge_index, mask, W1, a_src1, a_dst1, b1, W2, a_src2,
           a_dst2, b2, Wl, bl):
    features = np.asarray(features, np.float32)
    edge_index = np.asarray(edge_index, np.int32)
    mask = np.asarray(mask, np.int32)
    W1, W2, Wl = (np.asarray(a, np.float32) for a in (W1, W2, Wl))
    a_src1, a_dst1, b1 = (np.asarray(a, np.float32) for a in (a_src1, a_dst1, b1))
    a_src2, a_dst2, b2 = (np.asarray(a, np.float32) for a in (a_src2, a_dst2, b2))
    bl = np.asarray(bl, np.float32)
    wl0, wl1 = Wl[:F_IN, 0], Wl[F_IN:, 0]

    g = _prep_graph(edge_index)
    iota = np.ascontiguousarray(
        np.broadcast_to(np.arange(128, dtype=np.float32), (128, 128))
    )
    iotac = np.arange(128, dtype=np.float32).reshape(128, 1)

    key = (g["T"], tuple(int(x) for x in g["wt"]))
    if key not in _PROG_CACHE:
        _PROG_CACHE[key] = dict(
            p1=_build_proj(1, H, link_cols=False),
            a1=_build_agg(H, g, relu=True),
            az=_build_agg_z(g["wt"]),
            lk=_build_link2((10000 // NCORES + 127) // 128),
        )
    progs = _PROG_CACHE[key]

    # ---- L1: H1 = X @ W1 (sharded), es1/ed1
    xsh = []
    for c in range(NCORES):
        xs = np.zeros((NSP, F_IN), np.float16)
        xs[:NS] = features[c * NS:(c + 1) * NS]
        xsh.append(xs)
    xT1 = _tile_xT(xsh, 1, F_IN)
    W1h = W1.astype(np.float16)
    r1 = _run("p1", progs["p1"], [
        dict(xT=xT1[c], Wm=W1h, asr=_rep(a_src1), adr=_rep(a_dst1))
        for c in range(NCORES)
    ])
    H1e = np.concatenate([r1[c]["h16"][:NS] for c in range(NCORES)])   # [N, H+1] f16
    es1 = np.concatenate([r1[c]["cols"][:, 0::2].T.ravel()[:NS] for c in range(NCORES)])
    ed1 = np.concatenate([r1[c]["cols"][:, 1::2].T.ravel()[:NS] for c in range(NCORES)])

    # ---- L2: aggregate layer 1, then fused in-epilogue layer-2 projection:
    # cols = [es2 | ed2 | z0p | z1p] per local node (h1r never leaves device)
    b1r = _rep(b1)
    T1 = np.zeros((N, 384), np.float16)
    T1[:, 0:H + 1] = H1e
    vfold = np.stack([W2 @ a_src2, W2 @ a_dst2, W2 @ wl0, W2 @ wl1])   # [4, 256]
    vrep = np.ascontiguousarray(np.broadcast_to(
        vfold.reshape(1, 4 * H), (128, 4 * H))).astype(np.float16)
    ohs = []
    for c in range(NCORES):
        ohc = np.zeros((128, g["T"], 128), np.float16)
        pp, tt = np.nonzero(g["kind"][c] == 0)
        ohc[pp, tt, g["dstf"][c][pp, tt].astype(np.int64)] = 1.0
        ohs.append(ohc)
    ins2 = []
    for c in range(NCORES):
        esx, edx, ess, eds = _expand(es1, ed1, g, c)
        st = np.zeros((NSP, H + 1), np.float16)
        st[:NS] = H1e[c * NS:(c + 1) * NS]
        ins2.append(dict(tabA=T1[:NSPLIT], tabB=T1[NSPLIT:],
                         idxA=g["idxA"][c], idxB=g["idxB"][c],
                         selftab=st, vrep=vrep, oh=ohs[c], dstf=g["dstf"][c],
                         esx=esx, edx=edx, esself=ess, edself=eds,
                         iota=iota, iotac=iotac, br=b1r))
    r2 = _run("a1", progs["a1"], ins2)
    es2 = np.concatenate([r2[c]["cols"][:, 0::4].T.ravel()[:NS] for c in range(NCORES)])
    ed2 = np.concatenate([r2[c]["cols"][:, 1::4].T.ravel()[:NS] for c in range(NCORES)])
    z0p = np.concatenate([r2[c]["cols"][:, 2::4].T.ravel()[:NS] for c in range(NCORES)])
    z1p = np.concatenate([r2[c]["cols"][:, 3::4].T.ravel()[:NS] for c in range(NCORES)])

    # ---- L4: scalar aggregation -> z0/z1 per node
    c0 = float(b2 @ wl0)
    c1 = float(b2 @ wl1)
    c01 = np.ascontiguousarray(
        np.broadcast_to(np.array([c0, c1], np.float32), (128, 2))
    )
    ins4 = []
    for c in range(NCORES):
        esx, edx, ess, eds = _expand(es2, ed2, g, c)
        real = (g["kind"][c] == 0)
        zin = np.zeros((128, g["T"], 4), np.float16)
        zin[:, :, 0] = np.where(real, z0p[g["srcs"][c]], 0.0)
        zin[:, :, 1] = np.where(real, z1p[g["srcs"][c]], 0.0)
        zin[:, :, 2] = real.astype(np.float16)
        nid = np.arange(NSP)
        nglob = np.minimum(c * NS + nid, N - 1)
        valid = (nid < NS)
        selfz = np.zeros((128, W, 4), np.float16)
        selfz[:, :, 0] = np.where(valid, z0p[nglob], 0.0).reshape(W, 128).T
        selfz[:, :, 1] = np.where(valid, z1p[nglob], 0.0).reshape(W, 128).T
        selfz[:, :, 2] = valid.astype(np.float16).reshape(W, 128).T
        ins4.append(dict(zin=zin, selfz=selfz, oh=ohs[c], dstf=g["dstf"][c],
                         esx=esx, edx=edx, esself=ess, edself=eds,
                         iota=iota, iotac=iotac, c01=c01))
    r4 = _run("az", progs["az"], ins4)
    zoc = [r4[c]["zo"] for c in range(NCORES)]         # [128, 2W] f32 per core
    z0f = np.concatenate(
        [zoc[c][:, 0::2].T.ravel()[:NS] for c in range(NCORES)])
    z1f = np.concatenate(
        [zoc[c][:, 1::2].T.ravel()[:NS] for c in range(NCORES)])

    # ---- L5: z = sigmoid(z0[m0] + z1[m1] + bl)
    P = mask.shape[0]
    pc = P // NCORES
    pt = (pc + 127) // 128
    z0x = np.zeros((NCORES, 128, pt), np.float32)
    z1x = np.zeros((NCORES, 128, pt), np.float32)
    mT = mask.T
    for c in range(NCORES):
        s = np.arange(pc)
        z0x[c, s % 128, s // 128] = z0f[mT[0][c * pc:(c + 1) * pc]]
        z1x[c, s % 128, s // 128] = z1f[mT[1][c * pc:(c + 1) * pc]]
    blr = np.full((128, 1), float(bl[0]), np.float32)
    r5 = _run("lk", progs["lk"], [
        dict(z0x=z0x[c], z1x=z1x[c], blr=blr)
        for c in range(NCORES)
    ])
    out = np.zeros((P, 1), np.float32)
    for c in range(NCORES):
        s = np.arange(pc)
        out[c * pc:(c + 1) * pc, 0] = r5[c]["z"][s % 128, s // 128]

    tot = sum(v for v in LAST_EXEC_NS.values() if v)
    print(f"kernel launches ns: {LAST_EXEC_NS} total {tot}")
    return out


# revision 37
# speedup vs baseline: 1.1414x; 1.0064x over previous
"""Two-layer GAT (single-head, PyG-style) + link predictor on 8 TRN2 NeuronCores.

v2 strategy (memory-regime). Key observation: the second GAT layer's output h2
is consumed ONLY through four linear functionals per node — es2 = h2p@a_s2,
ed2 = h2p@a_d2, z0 = h2@wl0, z1 = h2@wl1 (wl0/wl1 = halves of the link weight
column). By linearity of the edge-softmax aggregation, layer-2 aggregation
therefore reduces to aggregating per-edge SCALARS:
    z0[dst] = (sum_e alpha_e * z0p[src_e]) / denom + b2.wl0
so launch 4 needs NO feature gather at all: the per-edge scalar operands
(z0p[src], z1p[src], 1) are host-expanded index-space arrays (same class as
the esx/edx expansions the v1 kernel already used), and the one-hot matmul
scatters them per destination window with a 4-wide rhs.

Launches:
  L1 p1 : h1p = X@W1 (+ folded es1/ed1 cols)        [unchanged from v1]
  L2 a1 : edge-softmax aggregate of h1p rows via per-tile indirect row
          gathers + one-hot PE matmuls; h1r = relu(agg+b1)   [unchanged]
  L3 p2 : h2p-projection folds extended to SIX columns:
          [W2 | W2@a_s2 | W2@a_d2 | W2@wl0 | W2@wl1]; outputs only the four
          per-node scalars (h2p itself never materializes in DRAM).
  L4 az : scalar aggregation (rhs = [z0x, z1x, 1, 0] per edge slot) ->
          z0/z1 per node. No gathers, PE cost (128+4) cols per tile.
  L5 lk : z = sigmoid(z0[m0] + z1[m1] + bl) on host-arranged index pairs.
"""
import time

import numpy as np

import concourse.bass as bass
import concourse.mybir as mybir
import concourse.tile as tile
from concourse import bacc
from concourse.bass_utils import run_bass_kernel_spmd

F32 = mybir.dt.float32
F16 = mybir.dt.float16
I32 = mybir.dt.int32

NCORES = 8
N, F_IN, H, C = 50000, 128, 256, 1
NS = N // NCORES            # 6250 nodes per shard
W = (NS + 127) // 128       # 49 windows per shard
NSP = W * 128               # 6272 padded slots
NEG = -1.0e30               # pad-edge sentinel (exp -> exactly 0)

LAST_EXEC_NS = {}           # launch name -> exec_time_ns (filled per kernel() call)
_PROG_CACHE = {}


# ----------------------------------------------------------------- host prep
NSPLIT = 32768          # int16 table split point for dma_gather
CH_TILES = 60           # max tiles per gather chunk (SBUF-bound, 3 bufs)


def _prep_graph(edge_index):
    """Partition non-self edges by dst shard; within each dst window, edges
    are segregated by src half (A: src<32768, B: src>=32768) into separate
    128-slot tiles so a chunked int16 dma_gather can fetch whole tile groups.
    Common (max-over-cores) per-window tile counts wtA/wtB keep the program
    SPMD-shared. Edge slot s of a window region is (t, p) = (s//128, s%128)."""
    src = np.asarray(edge_index[0], np.int64)
    dst = np.asarray(edge_index[1], np.int64)

    core = dst // NS
    order = np.argsort(dst, kind="stable")
    src, dst, core = src[order], dst[order], core[order]

    e_src, e_dstloc = [], []
    for c in range(NCORES):
        m = core == c
        e_src.append(src[m])
        e_dstloc.append(dst[m] - c * NS)

    wtA = np.zeros(W, dtype=np.int64)
    wtB = np.zeros(W, dtype=np.int64)
    for c in range(NCORES):
        win = e_dstloc[c] // 128
        isA = e_src[c] < NSPLIT
        cntA = np.bincount(win[isA], minlength=W)
        cntB = np.bincount(win[~isA], minlength=W)
        wtA = np.maximum(wtA, (cntA + 127) // 128)
        wtB = np.maximum(wtB, (cntB + 127) // 128)
    wt = wtA + wtB
    T = int(wt.sum())
    wstart = np.concatenate([[0], np.cumsum(wt)]).astype(np.int64)

    srcs = np.zeros((NCORES, 128, T), dtype=np.int32)
    dstg = np.zeros((NCORES, 128, T), dtype=np.int32)
    dstf = np.full((NCORES, 128, T), -1.0, dtype=np.float32)
    kind = np.ones((NCORES, 128, T), dtype=np.int8)      # 0 real 1 pad
    # default pad sources: A tiles -> row 0, B tiles -> row NSPLIT
    for w in range(W):
        t0 = int(wstart[w])
        srcs[:, :, t0 + int(wtA[w]):t0 + int(wt[w])] = NSPLIT

    for c in range(NCORES):
        win = e_dstloc[c] // 128
        isA = e_src[c] < NSPLIT
        for w in range(W):
            t0 = int(wstart[w])
            for half, toff, flag in ((0, t0, True), (1, t0 + int(wtA[w]), False)):
                m = (win == w) & (isA == flag)
                s = e_src[c][m]
                dl = e_dstloc[c][m]
                n_e = len(s)
                sl = np.arange(n_e)
                tt, pp = toff + sl // 128, sl % 128
                srcs[c, pp, tt] = s
                dstg[c, pp, tt] = (dl + c * NS).astype(np.int32)
                dstf[c, pp, tt] = (dl - 128 * w).astype(np.float32)
                kind[c, pp, tt] = 0

    # chunk consecutive windows for the gather calls
    chunks = []
    cur = []
    cur_tiles = 0
    for w in range(W):
        tw = int(wt[w])
        if cur and cur_tiles + tw > CH_TILES:
            chunks.append(cur)
            cur, cur_tiles = [], 0
        cur.append(w)
        cur_tiles += tw
    if cur:
        chunks.append(cur)
    # taper: split the last chunk small so the post-gather tail (serial
    # matmul+epilogue consumption of the final chunk) stays short
    if len(chunks) > 1:
        last = chunks.pop()
        cur, cur_tiles = [], 0
        for w in last:
            tw = int(wt[w])
            if cur and cur_tiles + tw > 20:
                chunks.append(cur)
                cur, cur_tiles = [], 0
            cur.append(w)
            cur_tiles += tw
        if cur:
            chunks.append(cur)

    # per-chunk tile lists (global tile ids) for A and B calls + per-tile blk
    chmeta = []          # (wins, tilesA, tilesB) ; blk of tile = position
    blk_of = np.zeros(T, dtype=np.int64)
    for wins in chunks:
        tA, tB = [], []
        for w in wins:
            t0 = int(wstart[w])
            tA.extend(range(t0, t0 + int(wtA[w])))
            tB.extend(range(t0 + int(wtA[w]), t0 + int(wt[w])))
        for i, t in enumerate(tA):
            blk_of[t] = i
        for i, t in enumerate(tB):
            blk_of[t] = len(tA) + i
        chmeta.append((wins, tA, tB))

    # host-built int16 gather index arrays (16-partition wrap, replicated x8)
    def wrap16(vals):
        nidx = len(vals)
        colsn = (nidx + 15) // 16
        arr = np.zeros((128, colsn), np.int16)
        s = np.arange(nidx)
        for grp in range(8):
            arr[grp * 16 + s % 16, s // 16] = vals
        return arr

    idxA, idxB = [], []
    for c in range(NCORES):
        pa, pb = [], []
        for wins, tA, tB in chmeta:
            if tA:
                pa.append(wrap16(srcs[c][:, tA].T.ravel()))
            if tB:
                pb.append(wrap16((srcs[c][:, tB].T.ravel() - NSPLIT)))
        idxA.append(np.concatenate(pa, axis=1) if pa else np.zeros((128, 1), np.int16))
        idxB.append(np.concatenate(pb, axis=1) if pb else np.zeros((128, 1), np.int16))

    return dict(srcs=srcs, dstg=dstg, dstf=dstf, kind=kind, wt=wt,
                wtA=wtA, wtB=wtB, T=T, wstart=wstart, chmeta=chmeta,
                blk_of=blk_of, idxA=idxA, idxB=idxB,
                scA=idxA[0].shape[1], scB=idxB[0].shape[1])


def _expand(es_full, ed_full, g, c):
    """Host halo expansion: per-edge es[src], ed[dst] (+sentinel for pads),
    and per-node self-loop es/ed in [128, W] layout."""
    esx = es_full[g["srcs"][c]].astype(np.float32)
    edx = ed_full[np.minimum(g["dstg"][c], N - 1)].astype(np.float32)
    pad = g["kind"][c] == 1
    esx[pad] = NEG
    edx[pad] = 0.0
    nid = np.arange(NSP)
    nglob = np.minimum(c * NS + nid, N - 1)
    ess = np.where(nid < NS, es_full[nglob], 0.0).astype(np.float32)
    eds = np.where(nid < NS, ed_full[nglob], 0.0).astype(np.float32)
    return esx, edx, ess.reshape(W, 128).T.copy(), eds.reshape(W, 128).T.copy()


# ------------------------------------------------------------- bass programs
def _build_proj(kc, d_out, link_cols):
    """Projection per 128-node window: psum = x @ [W | W@a_s | W@a_d (|W@wl0
    |W@wl1)].  Inputs: xT fp16 [kc, W, 128, 128], Wm fp16 [kc*128, d_out],
    asr/adr (w0r/w1r) fp32 [128, d_out].
    Outputs: h16 [NSP, d_out+1] fp16 (feat + 1.0 col; omitted if link_cols),
    es/ed (z0/z1) [128, W] f32."""
    nf = 4 if link_cols else 2
    nc = bacc.Bacc(num_devices=NCORES)
    xT = nc.dram_tensor("xT", [kc, W, 128, 128], F16, kind="ExternalInput").ap()
    Wm = nc.dram_tensor("Wm", [kc * 128, d_out], F16, kind="ExternalInput").ap()
    folds = []
    for j, nm in enumerate(["asr", "adr", "w0r", "w1r"][:nf]):
        folds.append(nc.dram_tensor(nm, [128, d_out], F32, kind="ExternalInput").ap())
    if not link_cols:
        h16 = nc.dram_tensor("h16", [NSP, d_out + 1], F16, kind="ExternalOutput").ap()
    cols = nc.dram_tensor("cols", [128, nf * W], F32, kind="ExternalOutput").ap()

    with tile.TileContext(nc) as tc:
        with (
            tc.tile_pool(name="const", bufs=1) as cpool,
            tc.tile_pool(name="x", bufs=6) as xpool,
            tc.tile_pool(name="o", bufs=4) as opool,
            tc.tile_pool(name="ps", bufs=4, space="PSUM") as pspool,
            tc.tile_pool(name="sc", bufs=4) as scpool,
        ):
            fsb = []
            for j in range(nf):
                fb = cpool.tile([128, d_out], F32, tag=f"f{j}")
                nc.sync.dma_start(out=fb[:], in_=folds[j])
                fsb.append(fb)
            obuf = cpool.tile([128, nf * W], F32, name="obuf")

            wsb = []
            for k in range(kc):
                wk = cpool.tile([128, d_out + nf], F16, tag=f"w{k}")
                nc.sync.dma_start(
                    out=wk[:, 0:d_out], in_=Wm[128 * k:128 * (k + 1), :]
                )
                for j in range(nf):
                    scr = scpool.tile([128, d_out], F32, tag=f"wf{j}")
                    nc.vector.tensor_tensor(
                        out=scr[:], in0=wk[:, 0:d_out], in1=fsb[j][:],
                        op=mybir.AluOpType.mult,
                    )
                    wcol = scpool.tile([128, 1], F32, tag=f"wc{j}")
                    nc.vector.reduce_sum(
                        out=wcol[:], in_=scr[:], axis=mybir.AxisListType.X
                    )
                    nc.vector.tensor_copy(
                        out=wk[:, d_out + j:d_out + j + 1], in_=wcol[:]
                    )
                wsb.append(wk)

            for w in range(W):
                ps = pspool.tile([128, d_out + nf], F32, space="PSUM")
                for k in range(kc):
                    xt = xpool.tile([128, 128], F16)
                    nc.sync.dma_start(out=xt[:], in_=xT[k, w])
                    nc.tensor.matmul(
                        out=ps[:], lhsT=xt[:], rhs=wsb[k][:],
                        start=(k == 0), stop=(k == kc - 1),
                    )
                if not link_cols:
                    ht = opool.tile([128, d_out + 1], F16)
                    nc.scalar.copy(out=ht[:, 0:d_out], in_=ps[:, 0:d_out])
                    nc.vector.memset(ht[:, d_out:d_out + 1], 1.0)
                    nc.sync.dma_start(out=h16[128 * w:128 * (w + 1), :], in_=ht[:])
                nc.vector.tensor_copy(
                    out=obuf[:, nf * w:nf * (w + 1)],
                    in_=ps[:, d_out:d_out + nf],
                )
            nc.sync.dma_start(out=cols[:], in_=obuf[:])
    nc.compile()
    return nc


def _softmax_weights(nc, cpool, es_t, ed_t, cols, tagp):
    lg = cpool.tile([128, cols], F32, tag=f"lg{tagp}", name=f"lg{tagp}")
    nc.vector.tensor_tensor(
        out=lg[:], in0=es_t[:], in1=ed_t[:], op=mybir.AluOpType.add
    )
    lg2 = cpool.tile([128, cols], F32, tag=f"lg2{tagp}", name=f"lg2{tagp}")
    nc.vector.tensor_scalar_mul(out=lg2[:], in0=lg[:], scalar1=0.2)
    nc.vector.tensor_tensor(
        out=lg[:], in0=lg[:], in1=lg2[:], op=mybir.AluOpType.max
    )
    p = cpool.tile([128, cols], F32, tag=f"p{tagp}", name=f"p{tagp}")
    nc.scalar.activation(
        out=p[:], in_=lg[:], func=mybir.ActivationFunctionType.Exp
    )
    return p


def _build_agg(d, g, relu):
    """Layer-1 aggregation. Source rows are fetched with chunked int16
    dma_gather calls (table split at NSPLIT; one call per chunk per half,
    ~7.4ns/row Q7 emission vs ~11ns/row for per-tile INDIRECT1D), then
    scattered per dst window with one-hot PE matmuls.
    Output ho: [NSP, d] fp16 (normalized aggregate + bias (+relu))."""
    wt, wtA, wstart = g["wt"], g["wtA"], g["wstart"]
    chmeta, blk_of = g["chmeta"], g["blk_of"]
    T = int(sum(wt))
    ELEM = 384
    nc = bacc.Bacc(num_devices=NCORES)
    tabA = nc.dram_tensor("tabA", [NSPLIT, ELEM], F16, kind="ExternalInput").ap()
    tabB = nc.dram_tensor("tabB", [N - NSPLIT, ELEM], F16, kind="ExternalInput").ap()
    idxA = nc.dram_tensor("idxA", [128, g["scA"]], mybir.dt.int16,
                          kind="ExternalInput").ap()
    idxB = nc.dram_tensor("idxB", [128, g["scB"]], mybir.dt.int16,
                          kind="ExternalInput").ap()
    selftab = nc.dram_tensor("selftab", [NSP, d + 1], F16, kind="ExternalInput").ap()
    vrep = nc.dram_tensor("vrep", [128, 4 * d], F16, kind="ExternalInput").ap()
    oh = nc.dram_tensor("oh", [128, T, 128], F16, kind="ExternalInput").ap()
    dstf = nc.dram_tensor("dstf", [128, T], F32, kind="ExternalInput").ap()
    esx = nc.dram_tensor("esx", [128, T], F32, kind="ExternalInput").ap()
    edx = nc.dram_tensor("edx", [128, T], F32, kind="ExternalInput").ap()
    esself = nc.dram_tensor("esself", [128, W], F32, kind="ExternalInput").ap()
    edself = nc.dram_tensor("edself", [128, W], F32, kind="ExternalInput").ap()
    iota = nc.dram_tensor("iota", [128, 128], F32, kind="ExternalInput").ap()
    iotac = nc.dram_tensor("iotac", [128, 1], F32, kind="ExternalInput").ap()
    br = nc.dram_tensor("br", [128, d], F32, kind="ExternalInput").ap()
    cols = nc.dram_tensor("cols", [128, 4 * W], F32, kind="ExternalOutput").ap()

    with tile.TileContext(nc) as tc:
        with (
            tc.tile_pool(name="const", bufs=1) as cpool,
            tc.tile_pool(name="g", bufs=3) as gpool,
            tc.tile_pool(name="sf", bufs=4) as sfpool,
            tc.tile_pool(name="s", bufs=8) as spool,
            tc.tile_pool(name="obh", bufs=3) as ohpool,
            tc.tile_pool(name="o", bufs=3) as opool,
            tc.tile_pool(name="cl", bufs=6) as clpool,
            tc.tile_pool(name="ps", bufs=4, space="PSUM") as pspool,
        ):
            idxAs = cpool.tile([128, g["scA"]], mybir.dt.int16)
            nc.sync.dma_start(out=idxAs[:], in_=idxA[:])
            idxBs = cpool.tile([128, g["scB"]], mybir.dt.int16)
            nc.sync.dma_start(out=idxBs[:], in_=idxB[:])
            dsts = cpool.tile([128, T], F32)
            nc.sync.dma_start(out=dsts[:], in_=dstf[:])
            esxs = cpool.tile([128, T], F32)
            nc.sync.dma_start(out=esxs[:], in_=esx[:])
            edxs = cpool.tile([128, T], F32)
            nc.sync.dma_start(out=edxs[:], in_=edx[:])
            esss = cpool.tile([128, W], F32)
            nc.sync.dma_start(out=esss[:], in_=esself[:])
            edss = cpool.tile([128, W], F32)
            nc.sync.dma_start(out=edss[:], in_=edself[:])
            iosb = cpool.tile([128, 128], F32)
            nc.sync.dma_start(out=iosb[:], in_=iota[:])
            iocs = cpool.tile([128, 1], F32)
            nc.sync.dma_start(out=iocs[:], in_=iotac[:])
            brs = cpool.tile([128, d], F32)
            nc.sync.dma_start(out=brs[:], in_=br[:])
            vreps = cpool.tile([128, 4 * d], F16)
            nc.sync.dma_start(out=vreps[:], in_=vrep[:])
            colsb = cpool.tile([128, 4 * W], F32)

            p_all = _softmax_weights(nc, cpool, esxs, edxs, T, "e")
            p_self = _softmax_weights(nc, cpool, esss, edss, W, "s")

            caoff, cboff = 0, 0
            ohb, ohb_base = None, -10
            for wins, tA, tB in chmeta:
                nA, nB = len(tA), len(tB)
                gbuf = gpool.tile([128, nA + nB, ELEM], F16, tag="gb")
                if nA:
                    nc.gpsimd.dma_gather(
                        gbuf[:, 0:nA, :], tabA[:], idxAs[:, caoff:caoff + nA * 8],
                        nA * 128, nA * 128, ELEM, single_packet=False,
                    )
                    caoff += nA * 8
                if nB:
                    nc.gpsimd.dma_gather(
                        gbuf[:, nA:nA + nB, :], tabB[:],
                        idxBs[:, cboff:cboff + nB * 8],
                        nB * 128, nB * 128, ELEM, single_packet=False,
                    )
                    cboff += nB * 8
                for w in wins:
                    t = int(wstart[w])
                    ps = pspool.tile([128, d + 1], F32, space="PSUM")
                    st = sfpool.tile([128, d + 1], F16)
                    nc.sync.dma_start(
                        out=st[:], in_=selftab[128 * w:128 * (w + 1), :]
                    )
                    sd = spool.tile([128, 128], F16, tag="sdiag")
                    nc.vector.scalar_tensor_tensor(
                        out=sd[:], in0=iosb[:], scalar=iocs[:, :1],
                        in1=p_self[:, w:w + 1].to_broadcast([128, 128]),
                        op0=mybir.AluOpType.is_equal, op1=mybir.AluOpType.mult,
                    )
                    nc.tensor.matmul(
                        out=ps[:], lhsT=sd[:], rhs=st[:],
                        start=True, stop=(int(wt[w]) == 0),
                    )
                    for i in range(int(wt[w])):
                        blk = int(blk_of[t])
                        # one-hot lhsT on ACT from static 0/1 tiles so the
                        # Vector engine only runs epilogues (keeps the
                        # sp -> matmul -> buffer-free -> gather chain clear)
                        if ohb is None or t >= ohb_base + 8:
                            ohb = ohpool.tile([128, 8, 128], F16, tag="ohb")
                            nb = min(8, T - t)
                            nc.sync.dma_start(
                                out=ohb[:, 0:nb, :], in_=oh[:, t:t + nb, :]
                            )
                            ohb_base = t
                        sp = spool.tile([128, 128], F16, tag="sedge")
                        nc.scalar.mul(
                            out=sp[:], in_=ohb[:, t - ohb_base, :],
                            mul=p_all[:, t:t + 1],
                        )
                        nc.tensor.matmul(
                            out=ps[:], lhsT=sp[:], rhs=gbuf[:, blk, 0:d + 1],
                            start=False, stop=(i == int(wt[w]) - 1),
                        )
                        t += 1
                    _agg_epilogue(nc, clpool, opool, ps, brs, vreps, colsb, d, w)
            nc.sync.dma_start(out=cols[:], in_=colsb[:])
    nc.compile()
    return nc


def _agg_epilogue(nc, clpool, opool, ps, brs, vreps, colsb, d, w):
    """h1r = relu(agg/denom + b1); then the fused layer-2 projection columns
    es2/ed2/z0p/z1p = h1r . (W2@a_s2 | W2@a_d2 | W2@wl0 | W2@wl1)."""
    rec = clpool.tile([128, 1], F32)
    nc.vector.reciprocal(rec[:], ps[:, d:d + 1])
    ot = opool.tile([128, d], F32)
    nc.vector.scalar_tensor_tensor(
        out=ot[:], in0=ps[:, 0:d], scalar=rec[:, :1], in1=brs[:],
        op0=mybir.AluOpType.mult, op1=mybir.AluOpType.add,
    )
    ot16 = opool.tile([128, d], F16, tag="o16")
    nc.vector.tensor_scalar_max(out=ot16[:], in0=ot[:], scalar1=0.0)
    for j in range(4):
        scr = opool.tile([128, d], F16, tag=f"scr{j}")
        nc.vector.tensor_tensor(
            out=scr[:], in0=ot16[:], in1=vreps[:, d * j:d * (j + 1)],
            op=mybir.AluOpType.mult,
        )
        nc.vector.reduce_sum(
            out=colsb[:, 4 * w + j:4 * w + j + 1], in_=scr[:],
            axis=mybir.AxisListType.X,
        )


def _build_agg_z(wt):
    """Layer-2 scalar aggregation: gather-free. Per edge slot the rhs operands
    are host-expanded scalars [z0p[src], z1p[src], 1, 0]; the one-hot matmul
    scatters them by dst; epilogue: z_j[dst] = ps[:, j]/ps[:, 2] + c_j.
    One-hot lhsT tiles alternate between DVE is_eq builds and ACT-scaled
    static 0/1 tiles streamed from DRAM, halving the per-tile serial cost."""
    T = int(sum(wt))
    nc = bacc.Bacc(num_devices=NCORES)
    zin = nc.dram_tensor("zin", [128, T, 4], F16, kind="ExternalInput").ap()
    selfz = nc.dram_tensor("selfz", [128, W, 4], F16, kind="ExternalInput").ap()
    oh = nc.dram_tensor("oh", [128, T, 128], F16, kind="ExternalInput").ap()
    dstf = nc.dram_tensor("dstf", [128, T], F32, kind="ExternalInput").ap()
    esx = nc.dram_tensor("esx", [128, T], F32, kind="ExternalInput").ap()
    edx = nc.dram_tensor("edx", [128, T], F32, kind="ExternalInput").ap()
    esself = nc.dram_tensor("esself", [128, W], F32, kind="ExternalInput").ap()
    edself = nc.dram_tensor("edself", [128, W], F32, kind="ExternalInput").ap()
    iota = nc.dram_tensor("iota", [128, 128], F32, kind="ExternalInput").ap()
    iotac = nc.dram_tensor("iotac", [128, 1], F32, kind="ExternalInput").ap()
    c01 = nc.dram_tensor("c01", [128, 2], F32, kind="ExternalInput").ap()
    zo = nc.dram_tensor("zo", [128, 2 * W], F32, kind="ExternalOutput").ap()

    with tile.TileContext(nc) as tc:
        with (
            tc.tile_pool(name="const", bufs=1) as cpool,
            tc.tile_pool(name="s", bufs=8) as spool,
            tc.tile_pool(name="obh", bufs=3) as ohpool,
            tc.tile_pool(name="cl", bufs=6) as clpool,
            tc.tile_pool(name="ps", bufs=4, space="PSUM") as pspool,
        ):
            zins = cpool.tile([128, T, 4], F16)
            nc.sync.dma_start(out=zins[:], in_=zin[:])
            selfzs = cpool.tile([128, W, 4], F16)
            nc.sync.dma_start(out=selfzs[:], in_=selfz[:])
            dsts = cpool.tile([128, T], F32)
            nc.sync.dma_start(out=dsts[:], in_=dstf[:])
            esxs = cpool.tile([128, T], F32)
            nc.sync.dma_start(out=esxs[:], in_=esx[:])
            edxs = cpool.tile([128, T], F32)
            nc.sync.dma_start(out=edxs[:], in_=edx[:])
            esss = cpool.tile([128, W], F32)
            nc.sync.dma_start(out=esss[:], in_=esself[:])
            edss = cpool.tile([128, W], F32)
            nc.sync.dma_start(out=edss[:], in_=edself[:])
            iosb = cpool.tile([128, 128], F32)
            nc.sync.dma_start(out=iosb[:], in_=iota[:])
            iocs = cpool.tile([128, 1], F32)
            nc.sync.dma_start(out=iocs[:], in_=iotac[:])
            c01s = cpool.tile([128, 2], F32)
            nc.sync.dma_start(out=c01s[:], in_=c01[:])
            zob = cpool.tile([128, 2 * W], F32)

            p_all = _softmax_weights(nc, cpool, esxs, edxs, T, "e")
            p_self = _softmax_weights(nc, cpool, esss, edss, W, "s")

            t = 0
            ohb, ohb_base = None, -1
            for w in range(W):
                ps = pspool.tile([128, 4], F32, space="PSUM")
                sd = spool.tile([128, 128], F16, tag="sdiag")
                nc.vector.scalar_tensor_tensor(
                    out=sd[:], in0=iosb[:], scalar=iocs[:, :1],
                    in1=p_self[:, w:w + 1].to_broadcast([128, 128]),
                    op0=mybir.AluOpType.is_equal, op1=mybir.AluOpType.mult,
                )
                nc.tensor.matmul(
                    out=ps[:], lhsT=sd[:], rhs=selfzs[:, w, 0:4],
                    start=True, stop=(int(wt[w]) == 0),
                )
                for i in range(int(wt[w])):
                    if t % 3 == 0:
                        # ACT path: static 0/1 tile (batch-loaded) scaled by p
                        if ohb is None or t >= ohb_base + 8:
                            ohb = ohpool.tile([128, 8, 128], F16, tag="ohb")
                            nb = min(8, T - t)
                            nc.sync.dma_start(
                                out=ohb[:, 0:nb, :], in_=oh[:, t:t + nb, :]
                            )
                            ohb_base = t
                        sp = spool.tile([128, 128], F16, tag="sedge")
                        nc.scalar.mul(
                            out=sp[:], in_=ohb[:, t - ohb_base, :],
                            mul=p_all[:, t:t + 1],
                        )
                    else:
                        sp = spool.tile([128, 128], F16, tag="sedge")
                        nc.vector.scalar_tensor_tensor(
                            out=sp[:], in0=iosb[:], scalar=dsts[:, t:t + 1],
                            in1=p_all[:, t:t + 1].to_broadcast([128, 128]),
                            op0=mybir.AluOpType.is_equal, op1=mybir.AluOpType.mult,
                        )
                    nc.tensor.matmul(
                        out=ps[:], lhsT=sp[:], rhs=zins[:, t, 0:4],
                        start=False, stop=(i == int(wt[w]) - 1),
                    )
                    t += 1
                rec = clpool.tile([128, 1], F32)
                nc.vector.reciprocal(rec[:], ps[:, 2:3])
                nc.vector.scalar_tensor_tensor(
                    out=zob[:, 2 * w:2 * w + 2], in0=ps[:, 0:2],
                    scalar=rec[:, :1], in1=c01s[:, 0:2],
                    op0=mybir.AluOpType.mult, op1=mybir.AluOpType.add,
                )
            nc.sync.dma_start(out=zo[:], in_=zob[:])
    nc.compile()
    return nc


def _build_link2(pt):
    """z = sigmoid(z0x + z1x + bl) for pt*128 host-arranged pairs."""
    nc = bacc.Bacc(num_devices=NCORES)
    z0x = nc.dram_tensor("z0x", [128, pt], F32, kind="ExternalInput").ap()
    z1x = nc.dram_tensor("z1x", [128, pt], F32, kind="ExternalInput").ap()
    blr = nc.dram_tensor("blr", [128, 1], F32, kind="ExternalInput").ap()
    z = nc.dram_tensor("z", [128, pt], F32, kind="ExternalOutput").ap()

    with tile.TileContext(nc) as tc:
        with tc.tile_pool(name="c", bufs=1) as cpool:
            z0s = cpool.tile([128, pt], F32)
            nc.sync.dma_start(out=z0s[:], in_=z0x[:])
            z1s = cpool.tile([128, pt], F32)
            nc.sync.dma_start(out=z1s[:], in_=z1x[:])
            bls = cpool.tile([128, 1], F32)
            nc.sync.dma_start(out=bls[:], in_=blr[:])
            zs = cpool.tile([128, pt], F32, name="zs")
            nc.vector.tensor_tensor(
                out=zs[:], in0=z0s[:], in1=z1s[:], op=mybir.AluOpType.add
            )
            zsb = cpool.tile([128, pt], F32, name="zsb")
            nc.scalar.activation(
                out=zsb[:], in_=zs[:],
                func=mybir.ActivationFunctionType.Sigmoid, bias=bls[:, :1],
            )
            nc.sync.dma_start(out=z[:], in_=zsb[:])
    nc.compile()
    return nc


def _run(name, nc, in_maps, trace=True):
    last = None
    for attempt in range(3):
        try:
            res = run_bass_kernel_spmd(
                nc, in_maps, core_ids=list(range(NCORES)), trace=trace
            )
            LAST_EXEC_NS[name] = res.exec_time_ns
            return res.results
        except Exception as e:  # wedged-device retry (clears on re-attempt)
            last = e
            time.sleep(5)
    raise last


def _rep(v, n=128):
    return np.ascontiguousarray(np.broadcast_to(np.asarray(v, np.float32), (n, len(v))))


def _tile_xT(xfull_shards, kc, d_in):
    """list of [NSP, d_in] fp16 per core -> [NCORES, kc, W, 128, 128] fp16."""
    out = np.zeros((NCORES, kc, W, 128, 128), np.float16)
    for c in range(NCORES):
        xt = xfull_shards[c].T  # [d_in, NSP]
        for k in range(kc):
            blk = xt[128 * k:128 * (k + 1)].reshape(128, W, 128)
            out[c, k] = np.transpose(blk, (1, 0, 2))
    return out


# ------------------------------------------------------------------- kernel
def kernel(features, edge_index, mask, W1, a_src1, a_dst1, b1, W2, a_src2,
           a_dst2, b2, Wl, bl):
    features = np.asarray(features, np.float32)
    edge_index = np.asarray(edge_index, np.int32)
    mask = np.asarray(mask, np.int32)
    W1, W2, Wl = (np.asarray(a, np.float32) for a in (W1, W2, Wl))
    a_src1, a_dst1, b1 = (np.asarray(a, np.float32) for a in (a_src1, a_dst1, b1))
    a_src2, a_dst2, b2 = (np.asarray(a, np.float32) for a in (a_src2, a_dst2, b2))
    bl = np.asarray(bl, np.float32)
    wl0, wl1 = Wl[:F_IN, 0], Wl[F_IN:, 0]

    g = _prep_graph(edge_index)
    iota = np.ascontiguousarray(
        np.broadcast_to(np.arange(128, dtype=np.float32), (128, 128))
    )
    iotac = np.arange(128, dtype=np.float32).reshape(128, 1)

    key = (g["T"], tuple(int(x) for x in g["wt"]))
    if key not in _PROG_CACHE:
        _PROG_CACHE[key] = dict(
            p1=_build_proj(1, H, link_cols=False),
            a1=_build_agg(H, g, relu=True),
            az=_build_agg_z(g["wt"]),
            lk=_build_link2((10000 // NCORES + 127) // 128),
        )
    progs = _PROG_CACHE[key]

    # ---- L1: H1 = X @ W1 (sharded), es1/ed1
    xsh = []
    for c in range(NCORES):
        xs = np.zeros((NSP, F_IN), np.float16)
        xs[:NS] = features[c * NS:(c + 1) * NS]
        xsh.append(xs)
    xT1 = _tile_xT(xsh, 1, F_IN)
    W1h = W1.astype(np.float16)
    r1 = _run("p1", progs["p1"], [
        dict(xT=xT1[c], Wm=W1h, asr=_rep(a_src1), adr=_rep(a_dst1))
        for c in range(NCORES)
    ])
    H1e = np.concatenate([r1[c]["h16"][:NS] for c in range(NCORES)])   # [N, H+1] f16
    es1 = np.concatenate([r1[c]["cols"][:, 0::2].T.ravel()[:NS] for c in range(NCORES)])
    ed1 = np.concatenate([r1[c]["cols"][:, 1::2].T.ravel()[:NS] for c in range(NCORES)])

    # ---- L2: aggregate layer 1, then fused in-epilogue layer-2 projection:
    # cols = [es2 | ed2 | z0p | z1p] per local node (h1r never leaves device)
    b1r = _rep(b1)
    T1 = np.zeros((N, 384), np.float16)
    T1[:, 0:H + 1] = H1e
    vfold = np.stack([W2 @ a_src2, W2 @ a_dst2, W2 @ wl0, W2 @ wl1])   # [4, 256]
    vrep = np.ascontiguousarray(np.broadcast_to(
        vfold.reshape(1, 4 * H), (128, 4 * H))).astype(np.float16)
    ohs = []
    for c in range(NCORES):
        ohc = np.zeros((128, g["T"], 128), np.float16)
        pp, tt = np.nonzero(g["kind"][c] == 0)
        ohc[pp, tt, g["dstf"][c][pp, tt].astype(np.int64)] = 1.0
        ohs.append(ohc)
    ins2 = []
    for c in range(NCORES):
        esx, edx, ess, eds = _expand(es1, ed1, g, c)
        st = np.zeros((NSP, H + 1), np.float16)
        st[:NS] = H1e[c * NS:(c + 1) * NS]
        ins2.append(dict(tabA=T1[:NSPLIT], tabB=T1[NSPLIT:],
                         idxA=g["idxA"][c], idxB=g["idxB"][c],
                         selftab=st, vrep=vrep, oh=ohs[c], dstf=g["dstf"][c],
                         esx=esx, edx=edx, esself=ess, edself=eds,
                         iota=iota, iotac=iotac, br=b1r))
    r2 = _run("a1", progs["a1"], ins2)
    es2 = np.concatenate([r2[c]["cols"][:, 0::4].T.ravel()[:NS] for c in range(NCORES)])
    ed2 = np.concatenate([r2[c]["cols"][:, 1::4].T.ravel()[:NS] for c in range(NCORES)])
    z0p = np.concatenate([r2[c]["cols"][:, 2::4].T.ravel()[:NS] for c in range(NCORES)])
    z1p = np.concatenate([r2[c]["cols"][:, 3::4].T.ravel()[:NS] for c in range(NCORES)])

    # ---- L4: scalar aggregation -> z0/z1 per node
    c0 = float(b2 @ wl0)
    c1 = float(b2 @ wl1)
    c01 = np.ascontiguousarray(
        np.broadcast_to(np.array([c0, c1], np.float32), (128, 2))
    )
    ins4 = []
    for c in range(NCORES):
        esx, edx, ess, eds = _expand(es2, ed2, g, c)
        real = (g["kind"][c] == 0)
        zin = np.zeros((128, g["T"], 4), np.float16)
        zin[:, :, 0] = np.where(real, z0p[g["srcs"][c]], 0.0)
        zin[:, :, 1] = np.where(real, z1p[g["srcs"][c]], 0.0)
        zin[:, :, 2] = real.astype(np.float16)
        nid = np.arange(NSP)
        nglob = np.minimum(c * NS + nid, N - 1)
        valid = (nid < NS)
        selfz = np.zeros((128, W, 4), np.float16)
        selfz[:, :, 0] = np.where(valid, z0p[nglob], 0.0).reshape(W, 128).T
        selfz[:, :, 1] = np.where(valid, z1p[nglob], 0.0).reshape(W, 128).T
        selfz[:, :, 2] = valid.astype(np.float16).reshape(W, 128).T
        ins4.append(dict(zin=zin, selfz=selfz, oh=ohs[c], dstf=g["dstf"][c],
                         esx=esx, edx=edx, esself=ess, edself=eds,
                         iota=iota, iotac=iotac, c01=c01))
    r4 = _run("az", progs["az"], ins4)
    zoc = [r4[c]["zo"] for c in range(NCORES)]         # [128, 2W] f32 per core
    z0f = np.concatenate(
        [zoc[c][:, 0::2].T.ravel()[:NS] for c in range(NCORES)])
    z1f = np.concatenate(
        [zoc[c][:, 1::2].T.ravel()[:NS] for c in range(NCORES)])

    # ---- L5: z = sigmoid(z0[m0] + z1[m1] + bl)
    P = mask.shape[0]
    pc = P // NCORES
    pt = (pc + 127) // 128
    z0x = np.zeros((NCORES, 128, pt), np.float32)
    z1x = np.zeros((NCORES, 128, pt), np.float32)
    mT = mask.T
    for c in range(NCORES):
        s = np.arange(pc)
        z0x[c, s % 128, s // 128] = z0f[mT[0][c * pc:(c + 1) * pc]]
        z1x[c, s % 128, s // 128] = z1f[mT[1][c * pc:(c + 1) * pc]]
    blr = np.full((128, 1), float(bl[0]), np.float32)
    r5 = _run("lk", progs["lk"], [
        dict(z0x=z0x[c], z1x=z1x[c], blr=blr)
        for c in range(NCORES)
    ])
    out = np.zeros((P, 1), np.float32)
    for c in range(NCORES):
        s = np.arange(pc)
        out[c * pc:(c + 1) * pc, 0] = r5[c]["z"][s % 128, s // 128]

    tot = sum(v for v in LAST_EXEC_NS.values() if v)
    print(f"kernel launches ns: {LAST_EXEC_NS} total {tot}")
    return out


# revision 38
# speedup vs baseline: 1.1952x; 1.0471x over previous
"""Two-layer GAT (single-head, PyG-style) + link predictor on 8 TRN2 NeuronCores.

v2 strategy (memory-regime). Key observation: the second GAT layer's output h2
is consumed ONLY through four linear functionals per node — es2 = h2p@a_s2,
ed2 = h2p@a_d2, z0 = h2@wl0, z1 = h2@wl1 (wl0/wl1 = halves of the link weight
column). By linearity of the edge-softmax aggregation, layer-2 aggregation
therefore reduces to aggregating per-edge SCALARS:
    z0[dst] = (sum_e alpha_e * z0p[src_e]) / denom + b2.wl0
so launch 4 needs NO feature gather at all: the per-edge scalar operands
(z0p[src], z1p[src], 1) are host-expanded index-space arrays (same class as
the esx/edx expansions the v1 kernel already used), and the one-hot matmul
scatters them per destination window with a 4-wide rhs.

Launches:
  L1 p1 : h1p = X@W1 (+ folded es1/ed1 cols)        [unchanged from v1]
  L2 a1 : edge-softmax aggregate of h1p rows via per-tile indirect row
          gathers + one-hot PE matmuls; h1r = relu(agg+b1)   [unchanged]
  L3 p2 : h2p-projection folds extended to SIX columns:
          [W2 | W2@a_s2 | W2@a_d2 | W2@wl0 | W2@wl1]; outputs only the four
          per-node scalars (h2p itself never materializes in DRAM).
  L4 az : scalar aggregation (rhs = [z0x, z1x, 1, 0] per edge slot) ->
          z0/z1 per node. No gathers, PE cost (128+4) cols per tile.
  L5 lk : z = sigmoid(z0[m0] + z1[m1] + bl) on host-arranged index pairs.
"""
import time

import numpy as np

import concourse.bass as bass
import concourse.mybir as mybir
import concourse.tile as tile
from concourse import bacc
from concourse.bass_utils import run_bass_kernel_spmd

F32 = mybir.dt.float32
F16 = mybir.dt.float16
I32 = mybir.dt.int32

NCORES = 8
N, F_IN, H, C = 50000, 128, 256, 1
NS = N // NCORES            # 6250 nodes per shard
W = (NS + 127) // 128       # 49 windows per shard
NSP = W * 128               # 6272 padded slots
NEG = -1.0e30               # pad-edge sentinel (exp -> exactly 0)

LAST_EXEC_NS = {}           # launch name -> exec_time_ns (filled per kernel() call)
_PROG_CACHE = {}


# ----------------------------------------------------------------- host prep
NSPLIT = 32768          # int16 table split point for dma_gather
CH_TILES = 46           # max tiles per gather chunk (SBUF-bound, 4 bufs)


def _prep_graph(edge_index):
    """Partition non-self edges by dst shard; within each dst window, edges
    are segregated by src half (A: src<32768, B: src>=32768) into separate
    128-slot tiles so a chunked int16 dma_gather can fetch whole tile groups.
    Common (max-over-cores) per-window tile counts wtA/wtB keep the program
    SPMD-shared. Edge slot s of a window region is (t, p) = (s//128, s%128)."""
    src = np.asarray(edge_index[0], np.int64)
    dst = np.asarray(edge_index[1], np.int64)

    core = dst // NS
    order = np.argsort(dst, kind="stable")
    src, dst, core = src[order], dst[order], core[order]

    e_src, e_dstloc = [], []
    for c in range(NCORES):
        m = core == c
        e_src.append(src[m])
        e_dstloc.append(dst[m] - c * NS)

    wtA = np.zeros(W, dtype=np.int64)
    wtB = np.zeros(W, dtype=np.int64)
    for c in range(NCORES):
        win = e_dstloc[c] // 128
        isA = e_src[c] < NSPLIT
        cntA = np.bincount(win[isA], minlength=W)
        cntB = np.bincount(win[~isA], minlength=W)
        wtA = np.maximum(wtA, (cntA + 127) // 128)
        wtB = np.maximum(wtB, (cntB + 127) // 128)
    wt = wtA + wtB
    T = int(wt.sum())
    wstart = np.concatenate([[0], np.cumsum(wt)]).astype(np.int64)

    srcs = np.zeros((NCORES, 128, T), dtype=np.int32)
    dstg = np.zeros((NCORES, 128, T), dtype=np.int32)
    dstf = np.full((NCORES, 128, T), -1.0, dtype=np.float32)
    kind = np.ones((NCORES, 128, T), dtype=np.int8)      # 0 real 1 pad
    # default pad sources: A tiles -> row 0, B tiles -> row NSPLIT
    for w in range(W):
        t0 = int(wstart[w])
        srcs[:, :, t0 + int(wtA[w]):t0 + int(wt[w])] = NSPLIT

    for c in range(NCORES):
        win = e_dstloc[c] // 128
        isA = e_src[c] < NSPLIT
        for w in range(W):
            t0 = int(wstart[w])
            for half, toff, flag in ((0, t0, True), (1, t0 + int(wtA[w]), False)):
                m = (win == w) & (isA == flag)
                s = e_src[c][m]
                dl = e_dstloc[c][m]
                n_e = len(s)
                sl = np.arange(n_e)
                tt, pp = toff + sl // 128, sl % 128
                srcs[c, pp, tt] = s
                dstg[c, pp, tt] = (dl + c * NS).astype(np.int32)
                dstf[c, pp, tt] = (dl - 128 * w).astype(np.float32)
                kind[c, pp, tt] = 0

    # chunk consecutive windows for the gather calls
    chunks = []
    cur = []
    cur_tiles = 0
    for w in range(W):
        tw = int(wt[w])
        if cur and cur_tiles + tw > CH_TILES:
            chunks.append(cur)
            cur, cur_tiles = [], 0
        cur.append(w)
        cur_tiles += tw
    if cur:
        chunks.append(cur)
    # taper: split the last chunk small so the post-gather tail (serial
    # matmul+epilogue consumption of the final chunk) stays short
    if len(chunks) > 1:
        last = chunks.pop()
        cur, cur_tiles = [], 0
        for w in last:
            tw = int(wt[w])
            if cur and cur_tiles + tw > 20:
                chunks.append(cur)
                cur, cur_tiles = [], 0
            cur.append(w)
            cur_tiles += tw
        if cur:
            chunks.append(cur)

    # per-chunk tile lists (global tile ids) for A and B calls + per-tile blk
    chmeta = []          # (wins, tilesA, tilesB) ; blk of tile = position
    blk_of = np.zeros(T, dtype=np.int64)
    for wins in chunks:
        tA, tB = [], []
        for w in wins:
            t0 = int(wstart[w])
            tA.extend(range(t0, t0 + int(wtA[w])))
            tB.extend(range(t0 + int(wtA[w]), t0 + int(wt[w])))
        for i, t in enumerate(tA):
            blk_of[t] = i
        for i, t in enumerate(tB):
            blk_of[t] = len(tA) + i
        chmeta.append((wins, tA, tB))

    # host-built int16 gather index arrays (16-partition wrap, replicated x8)
    def wrap16(vals):
        nidx = len(vals)
        colsn = (nidx + 15) // 16
        arr = np.zeros((128, colsn), np.int16)
        s = np.arange(nidx)
        for grp in range(8):
            arr[grp * 16 + s % 16, s // 16] = vals
        return arr

    idxA, idxB = [], []
    for c in range(NCORES):
        pa, pb = [], []
        for wins, tA, tB in chmeta:
            if tA:
                pa.append(wrap16(srcs[c][:, tA].T.ravel()))
            if tB:
                pb.append(wrap16((srcs[c][:, tB].T.ravel() - NSPLIT)))
        idxA.append(np.concatenate(pa, axis=1) if pa else np.zeros((128, 1), np.int16))
        idxB.append(np.concatenate(pb, axis=1) if pb else np.zeros((128, 1), np.int16))

    return dict(srcs=srcs, dstg=dstg, dstf=dstf, kind=kind, wt=wt,
                wtA=wtA, wtB=wtB, T=T, wstart=wstart, chmeta=chmeta,
                blk_of=blk_of, idxA=idxA, idxB=idxB,
                scA=idxA[0].shape[1], scB=idxB[0].shape[1])


def _expand(es_full, ed_full, g, c):
    """Host halo expansion: per-edge es[src], ed[dst] (+sentinel for pads),
    and per-node self-loop es/ed in [128, W] layout."""
    esx = es_full[g["srcs"][c]].astype(np.float32)
    edx = ed_full[np.minimum(g["dstg"][c], N - 1)].astype(np.float32)
    pad = g["kind"][c] == 1
    esx[pad] = NEG
    edx[pad] = 0.0
    nid = np.arange(NSP)
    nglob = np.minimum(c * NS + nid, N - 1)
    ess = np.where(nid < NS, es_full[nglob], 0.0).astype(np.float32)
    eds = np.where(nid < NS, ed_full[nglob], 0.0).astype(np.float32)
    return esx, edx, ess.reshape(W, 128).T.copy(), eds.reshape(W, 128).T.copy()


# ------------------------------------------------------------- bass programs
def _build_proj(kc, d_out, link_cols):
    """Projection per 128-node window: psum = x @ [W | W@a_s | W@a_d (|W@wl0
    |W@wl1)].  Inputs: xT fp16 [kc, W, 128, 128], Wm fp16 [kc*128, d_out],
    asr/adr (w0r/w1r) fp32 [128, d_out].
    Outputs: h16 [NSP, d_out+1] fp16 (feat + 1.0 col; omitted if link_cols),
    es/ed (z0/z1) [128, W] f32."""
    nf = 4 if link_cols else 2
    nc = bacc.Bacc(num_devices=NCORES)
    xT = nc.dram_tensor("xT", [kc, W, 128, 128], F16, kind="ExternalInput").ap()
    Wm = nc.dram_tensor("Wm", [kc * 128, d_out], F16, kind="ExternalInput").ap()
    folds = []
    for j, nm in enumerate(["asr", "adr", "w0r", "w1r"][:nf]):
        folds.append(nc.dram_tensor(nm, [128, d_out], F32, kind="ExternalInput").ap())
    if not link_cols:
        h16 = nc.dram_tensor("h16", [NSP, d_out + 1], F16, kind="ExternalOutput").ap()
    cols = nc.dram_tensor("cols", [128, nf * W], F32, kind="ExternalOutput").ap()

    with tile.TileContext(nc) as tc:
        with (
            tc.tile_pool(name="const", bufs=1) as cpool,
            tc.tile_pool(name="x", bufs=12) as xpool,
            tc.tile_pool(name="o", bufs=4) as opool,
            tc.tile_pool(name="ps", bufs=4, space="PSUM") as pspool,
            tc.tile_pool(name="sc", bufs=4) as scpool,
        ):
            fsb = []
            for j in range(nf):
                fb = cpool.tile([128, d_out], F32, tag=f"f{j}")
                nc.sync.dma_start(out=fb[:], in_=folds[j])
                fsb.append(fb)
            obuf = cpool.tile([128, nf * W], F32, name="obuf")

            wsb = []
            for k in range(kc):
                wk = cpool.tile([128, d_out + nf], F16, tag=f"w{k}")
                nc.sync.dma_start(
                    out=wk[:, 0:d_out], in_=Wm[128 * k:128 * (k + 1), :]
                )
                for j in range(nf):
                    scr = scpool.tile([128, d_out], F32, tag=f"wf{j}")
                    nc.vector.tensor_tensor(
                        out=scr[:], in0=wk[:, 0:d_out], in1=fsb[j][:],
                        op=mybir.AluOpType.mult,
                    )
                    wcol = scpool.tile([128, 1], F32, tag=f"wc{j}")
                    nc.vector.reduce_sum(
                        out=wcol[:], in_=scr[:], axis=mybir.AxisListType.X
                    )
                    nc.vector.tensor_copy(
                        out=wk[:, d_out + j:d_out + j + 1], in_=wcol[:]
                    )
                wsb.append(wk)

            for w in range(W):
                ps = pspool.tile([128, d_out + nf], F32, space="PSUM")
                for k in range(kc):
                    xt = xpool.tile([128, 128], F16)
                    nc.sync.dma_start(out=xt[:], in_=xT[k, w])
                    nc.tensor.matmul(
                        out=ps[:], lhsT=xt[:], rhs=wsb[k][:],
                        start=(k == 0), stop=(k == kc - 1),
                    )
                if not link_cols:
                    ht = opool.tile([128, d_out + 1], F16)
                    nc.scalar.copy(out=ht[:, 0:d_out], in_=ps[:, 0:d_out])
                    nc.vector.memset(ht[:, d_out:d_out + 1], 1.0)
                    nc.sync.dma_start(out=h16[128 * w:128 * (w + 1), :], in_=ht[:])
                nc.vector.tensor_copy(
                    out=obuf[:, nf * w:nf * (w + 1)],
                    in_=ps[:, d_out:d_out + nf],
                )
            nc.sync.dma_start(out=cols[:], in_=obuf[:])
    nc.compile()
    return nc


def _softmax_weights(nc, cpool, es_t, ed_t, cols, tagp):
    lg = cpool.tile([128, cols], F32, tag=f"lg{tagp}", name=f"lg{tagp}")
    nc.vector.tensor_tensor(
        out=lg[:], in0=es_t[:], in1=ed_t[:], op=mybir.AluOpType.add
    )
    lg2 = cpool.tile([128, cols], F32, tag=f"lg2{tagp}", name=f"lg2{tagp}")
    nc.vector.tensor_scalar_mul(out=lg2[:], in0=lg[:], scalar1=0.2)
    nc.vector.tensor_tensor(
        out=lg[:], in0=lg[:], in1=lg2[:], op=mybir.AluOpType.max
    )
    p = cpool.tile([128, cols], F32, tag=f"p{tagp}", name=f"p{tagp}")
    nc.scalar.activation(
        out=p[:], in_=lg[:], func=mybir.ActivationFunctionType.Exp
    )
    return p


def _build_agg(d, g, relu):
    """Layer-1 aggregation. Source rows are fetched with chunked int16
    dma_gather calls (table split at NSPLIT; one call per chunk per half,
    ~7.4ns/row Q7 emission vs ~11ns/row for per-tile INDIRECT1D), then
    scattered per dst window with one-hot PE matmuls.
    Output ho: [NSP, d] fp16 (normalized aggregate + bias (+relu))."""
    wt, wtA, wstart = g["wt"], g["wtA"], g["wstart"]
    chmeta, blk_of = g["chmeta"], g["blk_of"]
    T = int(sum(wt))
    ELEM = 384
    nc = bacc.Bacc(num_devices=NCORES)
    tabA = nc.dram_tensor("tabA", [NSPLIT, ELEM], F16, kind="ExternalInput").ap()
    tabB = nc.dram_tensor("tabB", [N - NSPLIT, ELEM], F16, kind="ExternalInput").ap()
    idxA = nc.dram_tensor("idxA", [128, g["scA"]], mybir.dt.int16,
                          kind="ExternalInput").ap()
    idxB = nc.dram_tensor("idxB", [128, g["scB"]], mybir.dt.int16,
                          kind="ExternalInput").ap()
    selftab = nc.dram_tensor("selftab", [NSP, d + 1], F16, kind="ExternalInput").ap()
    vrep = nc.dram_tensor("vrep", [128, 4 * d], F16, kind="ExternalInput").ap()
    oh = nc.dram_tensor("oh", [128, T, 128], F16, kind="ExternalInput").ap()
    dstf = nc.dram_tensor("dstf", [128, T], F32, kind="ExternalInput").ap()
    esx = nc.dram_tensor("esx", [128, T], F32, kind="ExternalInput").ap()
    edx = nc.dram_tensor("edx", [128, T], F32, kind="ExternalInput").ap()
    esself = nc.dram_tensor("esself", [128, W], F32, kind="ExternalInput").ap()
    edself = nc.dram_tensor("edself", [128, W], F32, kind="ExternalInput").ap()
    iota = nc.dram_tensor("iota", [128, 128], F32, kind="ExternalInput").ap()
    iotac = nc.dram_tensor("iotac", [128, 1], F32, kind="ExternalInput").ap()
    br = nc.dram_tensor("br", [128, d], F32, kind="ExternalInput").ap()
    cols = nc.dram_tensor("cols", [128, 4 * W], F32, kind="ExternalOutput").ap()

    with tile.TileContext(nc) as tc:
        with (
            tc.tile_pool(name="const", bufs=1) as cpool,
            tc.tile_pool(name="g", bufs=4) as gpool,
            tc.tile_pool(name="sf", bufs=4) as sfpool,
            tc.tile_pool(name="s", bufs=8) as spool,
            tc.tile_pool(name="obh", bufs=3) as ohpool,
            tc.tile_pool(name="o", bufs=3) as opool,
            tc.tile_pool(name="cl", bufs=6) as clpool,
            tc.tile_pool(name="ps", bufs=4, space="PSUM") as pspool,
        ):
            idxAs = cpool.tile([128, g["scA"]], mybir.dt.int16)
            nc.sync.dma_start(out=idxAs[:], in_=idxA[:])
            idxBs = cpool.tile([128, g["scB"]], mybir.dt.int16)
            nc.sync.dma_start(out=idxBs[:], in_=idxB[:])
            dsts = cpool.tile([128, T], F32)
            nc.sync.dma_start(out=dsts[:], in_=dstf[:])
            esxs = cpool.tile([128, T], F32)
            nc.sync.dma_start(out=esxs[:], in_=esx[:])
            edxs = cpool.tile([128, T], F32)
            nc.sync.dma_start(out=edxs[:], in_=edx[:])
            esss = cpool.tile([128, W], F32)
            nc.sync.dma_start(out=esss[:], in_=esself[:])
            edss = cpool.tile([128, W], F32)
            nc.sync.dma_start(out=edss[:], in_=edself[:])
            iosb = cpool.tile([128, 128], F32)
            nc.sync.dma_start(out=iosb[:], in_=iota[:])
            iocs = cpool.tile([128, 1], F32)
            nc.sync.dma_start(out=iocs[:], in_=iotac[:])
            brs = cpool.tile([128, d], F32)
            nc.sync.dma_start(out=brs[:], in_=br[:])
            vreps = cpool.tile([128, 4 * d], F16)
            nc.sync.dma_start(out=vreps[:], in_=vrep[:])
            colsb = cpool.tile([128, 4 * W], F32)

            p_all = _softmax_weights(nc, cpool, esxs, edxs, T, "e")
            p_self = _softmax_weights(nc, cpool, esss, edss, W, "s")

            caoff, cboff = 0, 0
            ohb, ohb_base = None, -10
            for wins, tA, tB in chmeta:
                nA, nB = len(tA), len(tB)
                gbuf = gpool.tile([128, nA + nB, ELEM], F16, tag="gb")
                if nA:
                    nc.gpsimd.dma_gather(
                        gbuf[:, 0:nA, :], tabA[:], idxAs[:, caoff:caoff + nA * 8],
                        nA * 128, nA * 128, ELEM, single_packet=False,
                    )
                    caoff += nA * 8
                if nB:
                    nc.gpsimd.dma_gather(
                        gbuf[:, nA:nA + nB, :], tabB[:],
                        idxBs[:, cboff:cboff + nB * 8],
                        nB * 128, nB * 128, ELEM, single_packet=False,
                    )
                    cboff += nB * 8
                for w in wins:
                    t = int(wstart[w])
                    ps = pspool.tile([128, d + 1], F32, space="PSUM")
                    st = sfpool.tile([128, d + 1], F16)
                    nc.sync.dma_start(
                        out=st[:], in_=selftab[128 * w:128 * (w + 1), :]
                    )
                    sd = spool.tile([128, 128], F16, tag="sdiag")
                    nc.vector.scalar_tensor_tensor(
                        out=sd[:], in0=iosb[:], scalar=iocs[:, :1],
                        in1=p_self[:, w:w + 1].to_broadcast([128, 128]),
                        op0=mybir.AluOpType.is_equal, op1=mybir.AluOpType.mult,
                    )
                    nc.tensor.matmul(
                        out=ps[:], lhsT=sd[:], rhs=st[:],
                        start=True, stop=(int(wt[w]) == 0),
                    )
                    for i in range(int(wt[w])):
                        blk = int(blk_of[t])
                        # one-hot lhsT on ACT from static 0/1 tiles so the
                        # Vector engine only runs epilogues (keeps the
                        # sp -> matmul -> buffer-free -> gather chain clear)
                        if ohb is None or t >= ohb_base + 8:
                            ohb = ohpool.tile([128, 8, 128], F16, tag="ohb")
                            nb = min(8, T - t)
                            nc.sync.dma_start(
                                out=ohb[:, 0:nb, :], in_=oh[:, t:t + nb, :]
                            )
                            ohb_base = t
                        sp = spool.tile([128, 128], F16, tag="sedge")
                        nc.scalar.mul(
                            out=sp[:], in_=ohb[:, t - ohb_base, :],
                            mul=p_all[:, t:t + 1],
                        )
                        nc.tensor.matmul(
                            out=ps[:], lhsT=sp[:], rhs=gbuf[:, blk, 0:d + 1],
                            start=False, stop=(i == int(wt[w]) - 1),
                        )
                        t += 1
                    _agg_epilogue(nc, clpool, opool, ps, brs, vreps, colsb, d, w)
            nc.sync.dma_start(out=cols[:], in_=colsb[:])
    nc.compile()
    return nc


def _agg_epilogue(nc, clpool, opool, ps, brs, vreps, colsb, d, w):
    """h1r = relu(agg/denom + b1); then the fused layer-2 projection columns
    es2/ed2/z0p/z1p = h1r . (W2@a_s2 | W2@a_d2 | W2@wl0 | W2@wl1)."""
    rec = clpool.tile([128, 1], F32)
    nc.vector.reciprocal(rec[:], ps[:, d:d + 1])
    ot = opool.tile([128, d], F32)
    nc.vector.scalar_tensor_tensor(
        out=ot[:], in0=ps[:, 0:d], scalar=rec[:, :1], in1=brs[:],
        op0=mybir.AluOpType.mult, op1=mybir.AluOpType.add,
    )
    ot16 = opool.tile([128, d], F16, tag="o16")
    nc.vector.tensor_scalar_max(out=ot16[:], in0=ot[:], scalar1=0.0)
    for j in range(4):
        scr = opool.tile([128, d], F16, tag=f"scr{j}")
        nc.vector.tensor_tensor(
            out=scr[:], in0=ot16[:], in1=vreps[:, d * j:d * (j + 1)],
            op=mybir.AluOpType.mult,
        )
        nc.vector.reduce_sum(
            out=colsb[:, 4 * w + j:4 * w + j + 1], in_=scr[:],
            axis=mybir.AxisListType.X,
        )


def _build_agg_z(wt):
    """Layer-2 scalar aggregation: gather-free. Per edge slot the rhs operands
    are host-expanded scalars [z0p[src], z1p[src], 1, 0]; the one-hot matmul
    scatters them by dst; epilogue: z_j[dst] = ps[:, j]/ps[:, 2] + c_j.
    One-hot lhsT tiles alternate between DVE is_eq builds and ACT-scaled
    static 0/1 tiles streamed from DRAM, halving the per-tile serial cost."""
    T = int(sum(wt))
    nc = bacc.Bacc(num_devices=NCORES)
    zin = nc.dram_tensor("zin", [128, T, 4], F16, kind="ExternalInput").ap()
    selfz = nc.dram_tensor("selfz", [128, W, 4], F16, kind="ExternalInput").ap()
    oh = nc.dram_tensor("oh", [128, T, 128], F16, kind="ExternalInput").ap()
    dstf = nc.dram_tensor("dstf", [128, T], F32, kind="ExternalInput").ap()
    esx = nc.dram_tensor("esx", [128, T], F32, kind="ExternalInput").ap()
    edx = nc.dram_tensor("edx", [128, T], F32, kind="ExternalInput").ap()
    esself = nc.dram_tensor("esself", [128, W], F32, kind="ExternalInput").ap()
    edself = nc.dram_tensor("edself", [128, W], F32, kind="ExternalInput").ap()
    iota = nc.dram_tensor("iota", [128, 128], F32, kind="ExternalInput").ap()
    iotac = nc.dram_tensor("iotac", [128, 1], F32, kind="ExternalInput").ap()
    c01 = nc.dram_tensor("c01", [128, 2], F32, kind="ExternalInput").ap()
    zo = nc.dram_tensor("zo", [128, 2 * W], F32, kind="ExternalOutput").ap()

    with tile.TileContext(nc) as tc:
        with (
            tc.tile_pool(name="const", bufs=1) as cpool,
            tc.tile_pool(name="s", bufs=8) as spool,
            tc.tile_pool(name="obh", bufs=3) as ohpool,
            tc.tile_pool(name="cl", bufs=6) as clpool,
            tc.tile_pool(name="ps", bufs=4, space="PSUM") as pspool,
        ):
            zins = cpool.tile([128, T, 4], F16)
            nc.sync.dma_start(out=zins[:], in_=zin[:])
            selfzs = cpool.tile([128, W, 4], F16)
            nc.sync.dma_start(out=selfzs[:], in_=selfz[:])
            dsts = cpool.tile([128, T], F32)
            nc.sync.dma_start(out=dsts[:], in_=dstf[:])
            esxs = cpool.tile([128, T], F32)
            nc.sync.dma_start(out=esxs[:], in_=esx[:])
            edxs = cpool.tile([128, T], F32)
            nc.sync.dma_start(out=edxs[:], in_=edx[:])
            esss = cpool.tile([128, W], F32)
            nc.sync.dma_start(out=esss[:], in_=esself[:])
            edss = cpool.tile([128, W], F32)
            nc.sync.dma_start(out=edss[:], in_=edself[:])
            iosb = cpool.tile([128, 128], F32)
            nc.sync.dma_start(out=iosb[:], in_=iota[:])
            iocs = cpool.tile([128, 1], F32)
            nc.sync.dma_start(out=iocs[:], in_=iotac[:])
            c01s = cpool.tile([128, 2], F32)
            nc.sync.dma_start(out=c01s[:], in_=c01[:])
            zob = cpool.tile([128, 2 * W], F32)

            p_all = _softmax_weights(nc, cpool, esxs, edxs, T, "e")
            p_self = _softmax_weights(nc, cpool, esss, edss, W, "s")

            t = 0
            ohb, ohb_base = None, -1
            for w in range(W):
                ps = pspool.tile([128, 4], F32, space="PSUM")
                sd = spool.tile([128, 128], F16, tag="sdiag")
                nc.vector.scalar_tensor_tensor(
                    out=sd[:], in0=iosb[:], scalar=iocs[:, :1],
                    in1=p_self[:, w:w + 1].to_broadcast([128, 128]),
                    op0=mybir.AluOpType.is_equal, op1=mybir.AluOpType.mult,
                )
                nc.tensor.matmul(
                    out=ps[:], lhsT=sd[:], rhs=selfzs[:, w, 0:4],
                    start=True, stop=(int(wt[w]) == 0),
                )
                for i in range(int(wt[w])):
                    if t % 3 == 0:
                        # ACT path: static 0/1 tile (batch-loaded) scaled by p
                        if ohb is None or t >= ohb_base + 8:
                            ohb = ohpool.tile([128, 8, 128], F16, tag="ohb")
                            nb = min(8, T - t)
                            nc.sync.dma_start(
                                out=ohb[:, 0:nb, :], in_=oh[:, t:t + nb, :]
                            )
                            ohb_base = t
                        sp = spool.tile([128, 128], F16, tag="sedge")
                        nc.scalar.mul(
                            out=sp[:], in_=ohb[:, t - ohb_base, :],
                            mul=p_all[:, t:t + 1],
                        )
                    else:
                        sp = spool.tile([128, 128], F16, tag="sedge")
                        nc.vector.scalar_tensor_tensor(
                            out=sp[:], in0=iosb[:], scalar=dsts[:, t:t + 1],
                            in1=p_all[:, t:t + 1].to_broadcast([128, 128]),
                            op0=mybir.AluOpType.is_equal, op1=mybir.AluOpType.mult,
                        )
                    nc.tensor.matmul(
                        out=ps[:], lhsT=sp[:], rhs=zins[:, t, 0:4],
                        start=False, stop=(i == int(wt[w]) - 1),
                    )
                    t += 1
                rec = clpool.tile([128, 1], F32)
                nc.vector.reciprocal(rec[:], ps[:, 2:3])
                nc.vector.scalar_tensor_tensor(
                    out=zob[:, 2 * w:2 * w + 2], in0=ps[:, 0:2],
                    scalar=rec[:, :1], in1=c01s[:, 0:2],
                    op0=mybir.AluOpType.mult, op1=mybir.AluOpType.add,
                )
            nc.sync.dma_start(out=zo[:], in_=zob[:])
    nc.compile()
    return nc


def _build_link2(pt):
    """z = sigmoid(z0x + z1x + bl) for pt*128 host-arranged pairs."""
    nc = bacc.Bacc(num_devices=NCORES)
    z0x = nc.dram_tensor("z0x", [128, pt], F32, kind="ExternalInput").ap()
    z1x = nc.dram_tensor("z1x", [128, pt], F32, kind="ExternalInput").ap()
    blr = nc.dram_tensor("blr", [128, 1], F32, kind="ExternalInput").ap()
    z = nc.dram_tensor("z", [128, pt], F32, kind="ExternalOutput").ap()

    with tile.TileContext(nc) as tc:
        with tc.tile_pool(name="c", bufs=1) as cpool:
            z0s = cpool.tile([128, pt], F32)
            nc.sync.dma_start(out=z0s[:], in_=z0x[:])
            z1s = cpool.tile([128, pt], F32)
            nc.sync.dma_start(out=z1s[:], in_=z1x[:])
            bls = cpool.tile([128, 1], F32)
            nc.sync.dma_start(out=bls[:], in_=blr[:])
            zs = cpool.tile([128, pt], F32, name="zs")
            nc.vector.tensor_tensor(
                out=zs[:], in0=z0s[:], in1=z1s[:], op=mybir.AluOpType.add
            )
            zsb = cpool.tile([128, pt], F32, name="zsb")
            nc.scalar.activation(
                out=zsb[:], in_=zs[:],
                func=mybir.ActivationFunctionType.Sigmoid, bias=bls[:, :1],
            )
            nc.sync.dma_start(out=z[:], in_=zsb[:])
    nc.compile()
    return nc


def _run(name, nc, in_maps, trace=True):
    last = None
    for attempt in range(3):
        try:
            res = run_bass_kernel_spmd(
                nc, in_maps, core_ids=list(range(NCORES)), trace=trace
            )
            LAST_EXEC_NS[name] = res.exec_time_ns
            return res.results
        except Exception as e:  # wedged-device retry (clears on re-attempt)
            last = e
            time.sleep(5)
    raise last


def _rep(v, n=128):
    return np.ascontiguousarray(np.broadcast_to(np.asarray(v, np.float32), (n, len(v))))


def _tile_xT(xfull_shards, kc, d_in):
    """list of [NSP, d_in] fp16 per core -> [NCORES, kc, W, 128, 128] fp16."""
    out = np.zeros((NCORES, kc, W, 128, 128), np.float16)
    for c in range(NCORES):
        xt = xfull_shards[c].T  # [d_in, NSP]
        for k in range(kc):
            blk = xt[128 * k:128 * (k + 1)].reshape(128, W, 128)
            out[c, k] = np.transpose(blk, (1, 0, 2))
    return out


# ------------------------------------------------------------------- kernel
def kernel(features, edge_index, mask, W1, a_src1, a_dst1, b1, W2, a_src2,
           a_dst2, b2, Wl, bl):
    features = np.asarray(features, np.float32)
    edge_index = np.asarray(edge_index, np.int32)
    mask = np.asarray(mask, np.int32)
    W1, W2, Wl = (np.asarray(a, np.float32) for a in (W1, W2, Wl))
    a_src1, a_dst1, b1 = (np.asarray(a, np.float32) for a in (a_src1, a_dst1, b1))
    a_src2, a_dst2, b2 = (np.asarray(a, np.float32) for a in (a_src2, a_dst2, b2))
    bl = np.asarray(bl, np.float32)
    wl0, wl1 = Wl[:F_IN, 0], Wl[F_IN:, 0]

    g = _prep_graph(edge_index)
    iota = np.ascontiguousarray(
        np.broadcast_to(np.arange(128, dtype=np.float32), (128, 128))
    )
    iotac = np.arange(128, dtype=np.float32).reshape(128, 1)

    key = (g["T"], tuple(int(x) for x in g["wt"]))
    if key not in _PROG_CACHE:
        _PROG_CACHE[key] = dict(
            p1=_build_proj(1, H, link_cols=False),
            a1=_build_agg(H, g, relu=True),
            az=_build_agg_z(g["wt"]),
            lk=_build_link2((10000 // NCORES + 127) // 128),
        )
    progs = _PROG_CACHE[key]

    # ---- L1: H1 = X @ W1 (sharded), es1/ed1
    xsh = []
    for c in range(NCORES):
        xs = np.zeros((NSP, F_IN), np.float16)
        xs[:NS] = features[c * NS:(c + 1) * NS]
        xsh.append(xs)
    xT1 = _tile_xT(xsh, 1, F_IN)
    W1h = W1.astype(np.float16)
    r1 = _run("p1", progs["p1"], [
        dict(xT=xT1[c], Wm=W1h, asr=_rep(a_src1), adr=_rep(a_dst1))
        for c in range(NCORES)
    ])
    H1e = np.concatenate([r1[c]["h16"][:NS] for c in range(NCORES)])   # [N, H+1] f16
    es1 = np.concatenate([r1[c]["cols"][:, 0::2].T.ravel()[:NS] for c in range(NCORES)])
    ed1 = np.concatenate([r1[c]["cols"][:, 1::2].T.ravel()[:NS] for c in range(NCORES)])

    # ---- L2: aggregate layer 1, then fused in-epilogue layer-2 projection:
    # cols = [es2 | ed2 | z0p | z1p] per local node (h1r never leaves device)
    b1r = _rep(b1)
    T1 = np.zeros((N, 384), np.float16)
    T1[:, 0:H + 1] = H1e
    vfold = np.stack([W2 @ a_src2, W2 @ a_dst2, W2 @ wl0, W2 @ wl1])   # [4, 256]
    vrep = np.ascontiguousarray(np.broadcast_to(
        vfold.reshape(1, 4 * H), (128, 4 * H))).astype(np.float16)
    ohs = []
    for c in range(NCORES):
        ohc = np.zeros((128, g["T"], 128), np.float16)
        pp, tt = np.nonzero(g["kind"][c] == 0)
        ohc[pp, tt, g["dstf"][c][pp, tt].astype(np.int64)] = 1.0
        ohs.append(ohc)
    ins2 = []
    for c in range(NCORES):
        esx, edx, ess, eds = _expand(es1, ed1, g, c)
        st = np.zeros((NSP, H + 1), np.float16)
        st[:NS] = H1e[c * NS:(c + 1) * NS]
        ins2.append(dict(tabA=T1[:NSPLIT], tabB=T1[NSPLIT:],
                         idxA=g["idxA"][c], idxB=g["idxB"][c],
                         selftab=st, vrep=vrep, oh=ohs[c], dstf=g["dstf"][c],
                         esx=esx, edx=edx, esself=ess, edself=eds,
                         iota=iota, iotac=iotac, br=b1r))
    r2 = _run("a1", progs["a1"], ins2)
    es2 = np.concatenate([r2[c]["cols"][:, 0::4].T.ravel()[:NS] for c in range(NCORES)])
    ed2 = np.concatenate([r2[c]["cols"][:, 1::4].T.ravel()[:NS] for c in range(NCORES)])
    z0p = np.concatenate([r2[c]["cols"][:, 2::4].T.ravel()[:NS] for c in range(NCORES)])
    z1p = np.concatenate([r2[c]["cols"][:, 3::4].T.ravel()[:NS] for c in range(NCORES)])

    # ---- L4: scalar aggregation -> z0/z1 per node
    c0 = float(b2 @ wl0)
    c1 = float(b2 @ wl1)
    c01 = np.ascontiguousarray(
        np.broadcast_to(np.array([c0, c1], np.float32), (128, 2))
    )
    ins4 = []
    for c in range(NCORES):
        esx, edx, ess, eds = _expand(es2, ed2, g, c)
        real = (g["kind"][c] == 0)
        zin = np.zeros((128, g["T"], 4), np.float16)
        zin[:, :, 0] = np.where(real, z0p[g["srcs"][c]], 0.0)
        zin[:, :, 1] = np.where(real, z1p[g["srcs"][c]], 0.0)
        zin[:, :, 2] = real.astype(np.float16)
        nid = np.arange(NSP)
        nglob = np.minimum(c * NS + nid, N - 1)
        valid = (nid < NS)
        selfz = np.zeros((128, W, 4), np.float16)
        selfz[:, :, 0] = np.where(valid, z0p[nglob], 0.0).reshape(W, 128).T
        selfz[:, :, 1] = np.where(valid, z1p[nglob], 0.0).reshape(W, 128).T
        selfz[:, :, 2] = valid.astype(np.float16).reshape(W, 128).T
        ins4.append(dict(zin=zin, selfz=selfz, oh=ohs[c], dstf=g["dstf"][c],
                         esx=esx, edx=edx, esself=ess, edself=eds,
                         iota=iota, iotac=iotac, c01=c01))
    r4 = _run("az", progs["az"], ins4)
    zoc = [r4[c]["zo"] for c in range(NCORES)]         # [128, 2W] f32 per core
    z0f = np.concatenate(
        [zoc[c][:, 0::2].T.ravel()[:NS] for c in range(NCORES)])
    z1f = np.concatenate(
        [zoc[c][:, 1::2].T.ravel()[:NS] for c in range(NCORES)])

    # ---- L5: z = sigmoid(z0[m0] + z1[m1] + bl)
    P = mask.shape[0]
    pc = P // NCORES
    pt = (pc + 127) // 128
    z0x = np.zeros((NCORES, 128, pt), np.float32)
    z1x = np.zeros((NCORES, 128, pt), np.float32)
    mT = mask.T
    for c in range(NCORES):
        s = np.arange(pc)
        z0x[c, s % 128, s // 128] = z0f[mT[0][c * pc:(c + 1) * pc]]
        z1x[c, s % 128, s // 128] = z1f[mT[1][c * pc:(c + 1) * pc]]
    blr = np.full((128, 1), float(bl[0]), np.float32)
    r5 = _run("lk", progs["lk"], [
        dict(z0x=z0x[c], z1x=z1x[c], blr=blr)
        for c in range(NCORES)
    ])
    out = np.zeros((P, 1), np.float32)
    for c in range(NCORES):
        s = np.arange(pc)
        out[c * pc:(c + 1) * pc, 0] = r5[c]["z"][s % 128, s // 128]

    tot = sum(v for v in LAST_EXEC_NS.values() if v)
    print(f"kernel launches ns: {LAST_EXEC_NS} total {tot}")
    return out
